# revision 52
# baseline (speedup 1.0000x reference)
"""Trainium2 Bass kernel for a 2-layer mean-aggregation GraphSAGE GNN.

Strategy (8 NeuronCores, SPMD):
  - Nodes are assigned to (core, tile, slot) with degree balancing; each core
    owns 49 tiles x 128 slots = 6272 dst nodes and the ~100k edges into them.
  - Layer 1: per edge-chunk (128 edges) dma_gather x[src] rows from HBM.
    One-hot R[e, d] = (iota == dstslot[e]) is built for a whole tile-group in
    ONE batched DVE is_equal (stride-0 broadcast AP on the dst-slot operand);
    S^T = sum_e M[e,f]^T R[e,d] accumulates on TensorE (PSUM); 1/deg is
    applied at the PSUM evict (rb broadcast multiply) -> mean^T.
    H^T = relu(W1_l @ mean^T + W1_r @ x^T + b1) via matmuls + fused ScalarE.
  - g = h @ W2_l^T computed per tile (node-major), written to DRAM and
    AllGather'd across cores (bf16, split lo/hi for overlap).
  - Layer 2 (node-major): psO[d,f] = sum_e R[e,d]^T g[src e] via lhsT=R;
    psR[d,f] = H W2_r^T + 1*b2^T (rank-1 bias matmul).  out = psO*rdeg + psR
    with the per-partition rdeg scale fused into the ScalarE evict.
Host does index-only preprocessing (permutation, edge chunking, 1/deg) and
the final unshard.
"""

import functools
import numpy as np

N_CORES = 8
TILES = 49  # tiles per core
TILE = 128
SHARD = TILES * TILE  # 6272
SUPER = 7  # tiles per supertile (gather-call granularity)
N_SUPER = TILES // SUPER  # 7
LO_SUPERS = 4  # supertiles in the "lo" AllGather split
LO_ROWS = LO_SUPERS * SUPER * TILE  # 3584
HI_ROWS = SHARD - LO_ROWS  # 2688
HI1_ROWS = 2 * SUPER * TILE  # 1792: supertiles 4-5, AllGather'd early
HI2_ROWS = HI_ROWS - HI1_ROWS  # 896: supertile 6, small tail AllGather
HI_SPLIT = LO_ROWS + HI1_ROWS  # 5376
SPLIT16 = 32768  # int16 index limit for layer-1 x gather


def _ceil_div(a, b):
    return -(-a // b)


def _wrap_idxs(idx_flat):
    """Wrap a flat int16 index list into the [128, n/16] dma_gather layout:
    index i lives at [i%16, i//16], replicated across the 8 groups of 16
    partitions."""
    n = len(idx_flat)
    assert n % 16 == 0
    w = np.asarray(idx_flat, np.int16).reshape(n // 16, 16).T  # [16, n/16]
    return np.tile(w, (8, 1))  # [128, n/16]


def _preprocess(x, edge_index, n_nodes):
    """Index-only host preprocessing: node permutation, per-core edge chunk
    streams for both layers, degree reciprocals.  Returns a dict of
    per-core/shared arrays plus layout metadata."""
    src = np.asarray(edge_index[0], np.int64)
    dst = np.asarray(edge_index[1], np.int64)
    E = src.shape[0]

    deg = np.bincount(dst, minlength=n_nodes).astype(np.int64)
    rdeg = (1.0 / np.maximum(deg, 1)).astype(np.float32)

    # Degree-balanced permutation: sort nodes by degree desc, deal round-robin
    # over the 392 global tiles; node -> (core, tile, slot).
    order = np.argsort(-deg, kind="stable")
    g_tile = np.empty(n_nodes, np.int64)   # global tile of node
    g_slot = np.empty(n_nodes, np.int64)   # slot within tile
    n_gtiles = N_CORES * TILES
    idx = np.arange(n_nodes)
    g_tile[order] = idx % n_gtiles
    g_slot[order] = idx // n_gtiles
    core_of = g_tile // TILES
    tile_of = g_tile % TILES
    row_of = tile_of * TILE + g_slot  # row within core shard [0, SHARD)

    e_core = core_of[dst]
    e_tile = tile_of[dst]
    e_slot = g_slot[dst]

    # Layer-1 groups: by src id vs int16 limit.
    l1_grp = (src >= SPLIT16).astype(np.int64)  # 0 = lo (idx=src), 1 = hi
    l1_idx = np.where(l1_grp == 0, src, src - SPLIT16)

    # Layer-2 groups: by gathered-g row (AllGather split layout).
    s_core = core_of[src]
    s_row = row_of[src]
    l2_grp = (s_row >= LO_ROWS).astype(np.int64)
    # hi rows live in gf_hi as [8 x HI1_ROWS] then [8 x HI2_ROWS]
    hi_idx = np.where(
        s_row < HI_SPLIT,
        s_core * HI1_ROWS + (s_row - LO_ROWS),
        N_CORES * HI1_ROWS + s_core * HI2_ROWS + (s_row - HI_SPLIT))
    l2_idx = np.where(l2_grp == 0, s_core * LO_ROWS + s_row, hi_idx)

    def build_layer(grp, gidx):
        """Compute per-(core,tile,group) edge lists; fixed chunk budgets CA/CB
        (max over all cores/tiles); build idx/dstslot streams in supertile
        gather-call order."""
        counts = np.zeros((N_CORES, TILES, 2), np.int64)
        np.add.at(counts, (e_core, e_tile, grp), 1)
        CA = int(_ceil_div(counts[:, :, 0].max(), TILE))
        CB = int(_ceil_div(counts[:, :, 1].max(), TILE))
        # bucket edges
        key = (e_core * TILES + e_tile) * 2 + grp
        eorder = np.argsort(key * (2 * E) + gidx, kind="stable")  # sorted by key then src for DMA locality
        sorted_key = key[eorder]
        starts = np.searchsorted(sorted_key, np.arange(N_CORES * TILES * 2))
        ends = np.searchsorted(sorted_key, np.arange(N_CORES * TILES * 2) + 1)

        NCHUNK = TILES * (CA + CB)
        idx_cols_per_chunk = TILE // 16  # 8
        idx_arr = np.zeros((N_CORES, 128, NCHUNK * idx_cols_per_chunk), np.int16)
        ds_arr = np.full((N_CORES, 128, NCHUNK), -1.0, np.float32)

        for c in range(N_CORES):
            flat_idx = np.zeros(NCHUNK * TILE, np.int16)
            gc = 0  # global chunk cursor within core stream
            for S in range(N_SUPER):
                for g in range(2):
                    nch = CA if g == 0 else CB
                    # per-tile sorted edge pools for this (supertile, group)
                    pools = []
                    for t0 in range(SUPER):
                        t = S * SUPER + t0
                        k = ((c * TILES + t) * 2) + g
                        es = eorder[starts[k]:ends[k]]
                        assert len(es) <= nch * TILE
                        pools.append([es, 0])  # (sorted-by-src edges, cursor)
                    # distribute per gather-call window so each SDMA engine
                    # reads a contiguous sorted src range (HBM row locality):
                    # engine of group-rel position P is P % 16.
                    g_nch = SUPER * nch
                    g_base = gc * TILE  # stream position of group start
                    for q0 in range(0, g_nch, 8):
                        w_ch = np.arange(q0, min(q0 + 8, g_nch))
                        P = (w_ch[:, None] * TILE
                             + np.arange(TILE)[None, :]).ravel()
                        tiles_of = (P // TILE) // nch
                        for t0 in np.unique(tiles_of):
                            Q = P[tiles_of == t0]
                            es, cur = pools[t0]
                            take = min(len(Q), len(es) - cur)
                            if take <= 0:
                                continue
                            Qf = Q[:take]
                            Qe = Qf[np.lexsort((Qf, Qf % 16))]
                            sel = es[cur:cur + take]
                            pools[t0][1] = cur + take
                            ap = g_base + Qe  # absolute stream positions
                            flat_idx[ap] = gidx[sel].astype(np.int16)
                            ds_arr[c, ap % 128, ap // 128] = e_slot[sel]
                    gc += g_nch
            idx_arr[c] = _wrap_idxs(flat_idx)
        return dict(CA=CA, CB=CB, idx=idx_arr, ds=ds_arr)

    l1 = build_layer(l1_grp, l1_idx)
    l2 = build_layer(l2_grp, l2_idx)

    # Per-core x^T in slot order (zeros for pad slots) + 1/deg layouts.
    import ml_dtypes
    din = x.shape[1]
    xT = np.zeros((N_CORES, din, SHARD), np.float32)
    xT[core_of, :, row_of] = np.asarray(x, np.float32)
    xT_bf = xT.astype(ml_dtypes.bfloat16)
    rbn = np.ones((N_CORES, SHARD), np.float32)
    rbn[core_of, row_of] = rdeg
    rb = np.ascontiguousarray(
        np.broadcast_to(rbn[:, None, :], (N_CORES, 128, SHARD))
    ).astype(ml_dtypes.bfloat16)  # [C, 128, SHARD] col-bcast for L1 evict
    rbT = np.ascontiguousarray(
        rbn.reshape(N_CORES, TILES, TILE).transpose(0, 2, 1)
    ).astype(np.float32)  # [C, 128(slot), TILES] per-partition for L2 evict

    meta = dict(l1=l1, l2=l2, xT=xT_bf, rb=rb, rbT=rbT,
                core_of=core_of, row_of=row_of)
    return meta


@functools.lru_cache(maxsize=2)
def _build_program(din, dh, dout, CA1, CB1, CA2, CB2, n_lo, n_hi,
                   do_cc=True, do_c=True, shared_g=True):
    """Build the SPMD Bass/Tile program.  All shapes static."""
    import concourse.bacc as bacc
    import concourse.mybir as mybir
    import concourse.tile as tile
    from concourse.library_config import mlp

    bf16 = mybir.dt.bfloat16
    f32 = mybir.dt.float32
    i16 = mybir.dt.int16

    NC1 = TILES * (CA1 + CB1)
    NC2 = TILES * (CA2 + CB2)
    W1 = NC1 * 8  # idx cols (TILE/16 per chunk)
    W2 = NC2 * 8
    CMAX = max(CA1, CB1, CA2, CB2)

    nc = bacc.Bacc("TRN2", target_bir_lowering=False, debug=False,
                   num_devices=N_CORES, num_swdge_queues=4)

    # ---- I/O tensors ----
    xg = nc.dram_tensor("xg", [n_lo + n_hi, din], bf16, kind="ExternalInput")
    xT_d = nc.dram_tensor("xT", [din, SHARD], bf16, kind="ExternalInput")
    idx1_d = nc.dram_tensor("idx1", [128, W1], i16, kind="ExternalInput")
    idx2_d = nc.dram_tensor("idx2", [128, W2], i16, kind="ExternalInput")
    ds1_d = nc.dram_tensor("ds1", [128, NC1], bf16, kind="ExternalInput")
    ds2_d = nc.dram_tensor("ds2", [128, NC2], bf16, kind="ExternalInput")
    rb_d = nc.dram_tensor("rb", [128, SHARD], bf16, kind="ExternalInput")
    rbT_d = nc.dram_tensor("rbT", [128, TILES], f32, kind="ExternalInput")
    w1lT_d = nc.dram_tensor("w1lT", [din, dh], bf16, kind="ExternalInput")
    w1rT_d = nc.dram_tensor("w1rT", [din, dh], bf16, kind="ExternalInput")
    w2lT_d = nc.dram_tensor("w2lT", [128, dh // 128, dout], bf16, kind="ExternalInput")
    w2rT_d = nc.dram_tensor("w2rT", [128, dh // 128, dout], bf16, kind="ExternalInput")
    b1_d = nc.dram_tensor("b1", [128, dh // 128], f32, kind="ExternalInput")
    b2r_d = nc.dram_tensor("b2r", [1, dout], bf16, kind="ExternalInput")
    ones_d = nc.dram_tensor("ones1", [1, 128], bf16, kind="ExternalInput")
    iota_d = nc.dram_tensor("iota", [128, CMAX, 128], bf16, kind="ExternalInput")
    outN_d = nc.dram_tensor("outN", [SHARD, dout], f32, kind="ExternalOutput")

    # internal DRAM
    gl_lo = nc.dram_tensor("gl_lo", [LO_ROWS, dout], bf16)
    gl_hi1 = nc.dram_tensor("gl_hi1", [HI1_ROWS, dout], bf16)
    gl_hi2 = nc.dram_tensor("gl_hi2", [HI2_ROWS, dout], bf16)
    _aspace = "Shared" if shared_g else None
    gf_lo = nc.dram_tensor("gf_lo", [N_CORES * LO_ROWS, dout], bf16,
                           addr_space=_aspace)
    gf_hi = nc.dram_tensor("gf_hi", [N_CORES * HI_ROWS, dout], bf16,
                           addr_space=_aspace)

    NH = dh // 128  # h halves (2)

    with tile.TileContext(nc) as tc:
        with (
            tc.tile_pool(name="per", bufs=1) as per,       # persistent SBUF
            tc.tile_pool(name="gath", bufs=3) as gpool,    # gather buffers
            tc.tile_pool(name="rt", bufs=3) as rpool,      # one-hot R tiles
            tc.tile_pool(name="mt", bufs=2) as mpool,      # meanT / evict tiles
            tc.tile_pool(name="stg", bufs=3) as spool,     # staging for DRAM writes
            tc.tile_pool(name="ps_seg", bufs=2, space="PSUM") as ps_seg,
            tc.tile_pool(name="ps_h", bufs=2, space="PSUM") as ps_h,
            tc.tile_pool(name="ps_g", bufs=2, space="PSUM") as ps_g,
        ):
            # ---- persistent loads ----
            xT = per.tile([din, SHARD], bf16)
            idx = per.tile([128, max(W1, W2)], i16)  # idx1, then idx2
            ds1 = per.tile([128, NC1], bf16)
            ds2 = per.tile([128, NC2], bf16)
            rb = per.tile([128, SHARD], bf16)
            rbT = per.tile([128, TILES], f32)
            w1lT = per.tile([din, dh], bf16)
            w1rT = per.tile([din, dh], bf16)
            w2lT = per.tile([128, NH, dout], bf16)
            w2rT = per.tile([128, NH, dout], bf16)
            b1 = per.tile([128, NH], f32)
            b2r = per.tile([1, dout], bf16)
            ones1 = per.tile([1, 128], bf16)
            iota = per.tile([128, CMAX, 128], bf16)
            HT = per.tile([128, NH, SHARD], bf16)

            for t_sb, t_dr in [(xT, xT_d), (ds1, ds1_d), (ds2, ds2_d),
                               (rb, rb_d),
                               (rbT, rbT_d), (w1lT, w1lT_d), (w1rT, w1rT_d),
                               (w2lT, w2lT_d), (w2rT, w2rT_d), (b1, b1_d),
                               (b2r, b2r_d), (ones1, ones_d), (iota, iota_d)]:
                nc.sync.dma_start(t_sb[:], t_dr[:])
            nc.sync.dma_start(idx[:, 0:W1], idx1_d[:])

            nc.gpsimd.load_library(mlp)

            xg_lo = xg[0:n_lo, :]
            xg_hi = xg[n_lo:n_lo + n_hi, :]

            # ================= Stage A: layer 1 + H + g =================
            a_bufs = {}

            def _issue_a(S):
                mA = gpool.tile([128, SUPER * CA1, din], bf16, tag="mA")
                mB = gpool.tile([128, SUPER * CB1, din], bf16, tag="mB")
                a_bufs[S] = (mA, mB)
                ca_cols = SUPER * CA1 * 8
                cb_cols = SUPER * CB1 * 8
                col0 = S * (ca_cols + cb_cols)
                for buf, nch, src_ap, c0 in [(mA, SUPER * CA1, xg_lo, col0),
                                             (mB, SUPER * CB1, xg_hi, col0 + ca_cols)]:
                    for q0 in range(0, nch, 8):
                        n = min(8, nch - q0)
                        nc.gpsimd.dma_gather(
                            buf[:, q0:q0 + n, :], src_ap,
                            idx[:, c0 + q0 * 8:c0 + (q0 + n) * 8],
                            n * TILE, n * TILE, din)

            rq1 = {}

            def _build_r1(t):
                S, t0 = divmod(t, SUPER)
                gc0 = S * SUPER * (CA1 + CB1)
                rs = []
                for g, (CC, base) in enumerate(
                        [(CA1, gc0), (CB1, gc0 + SUPER * CA1)]):
                    gcs = base + t0 * CC
                    R = rpool.tile([128, CC, 128], bf16,
                                   tag=("RA" if g == 0 else "RB"))
                    nc.vector.tensor_tensor(
                        R[:], iota[:, 0:CC, :],
                        ds1[:, gcs:gcs + CC].broadcast_to([128, CC, 128]),
                        mybir.AluOpType.is_equal)
                    rs.append(R)
                rq1[t] = rs

            import concourse.mybir as _mb

            def _issue_c(S, which, bufs_by_S):
                ca_cols = SUPER * CA2 * 8
                cb_cols = SUPER * CB2 * 8
                col0 = S * (ca_cols + cb_cols)
                if which == "A":
                    mA2 = gpool.tile([128, SUPER * CA2, dout], bf16, tag="mA")
                    bufs_by_S.setdefault(S, {})["A"] = mA2
                    nch, src_ap, c0, buf = SUPER * CA2, gf_lo[:], col0, mA2
                else:
                    mB2 = gpool.tile([128, SUPER * CB2, dout], bf16, tag="mB")
                    bufs_by_S.setdefault(S, {})["B"] = mB2
                    nch, src_ap, c0, buf = (SUPER * CB2, gf_hi[:],
                                            col0 + ca_cols, mB2)
                for q0 in range(0, nch, 8):
                    n = min(8, nch - q0)
                    nc.gpsimd.dma_gather(
                        buf[:, q0:q0 + n, :], src_ap,
                        idx[:, c0 + q0 * 8:c0 + (q0 + n) * 8],
                        n * TILE, n * TILE, dout)

            _c_bufs = {}
            _issue_a(0)
            _issue_a(1)
            _build_r1(0)
            _build_r1(1)
            for t in range(TILES):
                S, t0 = divmod(t, SUPER)
                if t0 == 0 and S + 2 < N_SUPER:
                    _issue_a(S + 2)
                if t0 == 0 and S == N_SUPER - 3 and do_cc:
                    # every stage-A gather is already issued (depth-2
                    # prefetch), so AG-lo here blocks nothing and its
                    # transfer hides behind the last three supertiles.
                    nc.gpsimd.collective_compute(
                        "AllGather", _mb.AluOpType.bypass,
                        replica_groups=[list(range(N_CORES))],
                        ins=[gl_lo.ap().opt()], outs=[gf_lo.ap().opt()])
                if t0 == 0 and S == N_SUPER - 1 and do_cc:
                    # gl_hi1 (supertiles 4-5) is complete; AllGather it while
                    # the last supertile computes, leaving only the small
                    # supertile-6 tail for the post-loop AG-hi2.
                    nc.gpsimd.collective_compute(
                        "AllGather", _mb.AluOpType.bypass,
                        replica_groups=[list(range(N_CORES))],
                        ins=[gl_hi1.ap().opt()],
                        outs=[gf_hi[0:N_CORES * HI1_ROWS, :].opt()])
                if t + 2 < TILES:
                    _build_r1(t + 2)
                mA, mB = a_bufs[S]
                # segment-sum split over two PSUM banks so consecutive
                # accumulating matmuls overlap (same-bank chains serialize).
                psS0 = ps_seg.tile([128, 128], f32, tag="psS0")
                psS1 = ps_seg.tile([128, 128], f32, tag="psS1")
                RA, RB = rq1.pop(t)
                mms = ([(mA, t0 * CA1 + k, RA, k) for k in range(CA1)]
                       + [(mB, t0 * CB1 + k, RB, k) for k in range(CB1)])
                banks = [psS0, psS1]
                nb = [sum(1 for i in range(len(mms)) if i % 2 == b)
                      for b in range(2)]
                cnt = [0, 0]
                for i, (buf, c, R, k) in enumerate(mms):
                    b = i % 2
                    nc.tensor.matmul(banks[b][:], lhsT=buf[:, c, :],
                                     rhs=R[:, k, :], start=(cnt[b] == 0),
                                     stop=(cnt[b] == nb[b] - 1))
                    cnt[b] += 1
                # meanT = (psS0 + psS1) * rdeg  (ACT evicts bank1, DVE fuses)
                s1 = mpool.tile([128, 128], f32, tag="s1")
                nc.scalar.activation(s1[:], psS1[:],
                                     mybir.ActivationFunctionType.Copy)
                ssum = mpool.tile([128, 128], f32, tag="ssum")
                nc.vector.tensor_tensor(ssum[:], psS0[:], s1[:],
                                        mybir.AluOpType.add)
                meanT = mpool.tile([128, 128], bf16, tag="meanT")
                nc.vector.tensor_tensor(
                    meanT[:], ssum[:], rb[:, t * TILE:(t + 1) * TILE],
                    mybir.AluOpType.mult)
                # H^T halves
                for j in range(NH):
                    psH = ps_h.tile([128, 128], f32, tag="psH")
                    nc.tensor.matmul(psH[:], lhsT=w1lT[:, j * 128:(j + 1) * 128],
                                     rhs=meanT[:], start=True, stop=False)
                    nc.tensor.matmul(psH[:], lhsT=w1rT[:, j * 128:(j + 1) * 128],
                                     rhs=xT[:, t * TILE:(t + 1) * TILE],
                                     start=False, stop=True)
                    nc.scalar.activation(HT[:, j, t * TILE:(t + 1) * TILE], psH[:],
                                         mybir.ActivationFunctionType.Relu,
                                         bias=b1[:, j:j + 1])
                # g tile (node-major)
                psG = ps_g.tile([128, 128], f32, tag="psG")
                for j in range(NH):
                    nc.tensor.matmul(psG[:], lhsT=HT[:, j, t * TILE:(t + 1) * TILE],
                                     rhs=w2lT[:, j, :], start=(j == 0),
                                     stop=(j == NH - 1))
                gT = spool.tile([128, dout], bf16, tag="gT")
                nc.scalar.activation(gT[:], psG[:],
                                     mybir.ActivationFunctionType.Copy)
                row = t * TILE
                if row < LO_ROWS:
                    dst = gl_lo[row:row + TILE, :]
                elif row < HI_SPLIT:
                    dst = gl_hi1[row - LO_ROWS:row - LO_ROWS + TILE, :]
                else:
                    dst = gl_hi2[row - HI_SPLIT:row - HI_SPLIT + TILE, :]
                nc.sync.dma_start(dst, gT[:])

            # idx buffer is free of layer-1 readers once stage-A gathers are
            # issued; load the layer-2 index stream (overlaps the AllGather).
            nc.sync.dma_start(idx[:, 0:W2], idx2_d[:])
            if do_c:
                _issue_c(0, "A", _c_bufs)
                _issue_c(1, "A", _c_bufs)

            # ================= Stage C: layer 2 (node-major) =================
            rq2 = {}

            def _build_r2(t):
                S, t0 = divmod(t, SUPER)
                gc0 = S * SUPER * (CA2 + CB2)
                rs = []
                for g, (CC, base) in enumerate(
                        [(CA2, gc0), (CB2, gc0 + SUPER * CA2)]):
                    gcs = base + t0 * CC
                    R = rpool.tile([128, CC, 128], bf16,
                                   tag=("RA2" if g == 0 else "RB2"))
                    nc.vector.tensor_tensor(
                        R[:], iota[:, 0:CC, :],
                        ds2[:, gcs:gcs + CC].broadcast_to([128, CC, 128]),
                        mybir.AluOpType.is_equal)
                    rs.append(R)
                rq2[t] = rs

            if do_cc:
                nc.gpsimd.collective_compute(
                    "AllGather", _mb.AluOpType.bypass,
                    replica_groups=[list(range(N_CORES))],
                    ins=[gl_hi2.ap().opt()],
                    outs=[gf_hi[N_CORES * HI1_ROWS:N_CORES * HI_ROWS, :].opt()])
            if do_c:
                _issue_c(2, "A", _c_bufs)
                _issue_c(0, "B", _c_bufs)
                _build_r2(0)
                _build_r2(1)
            for t in (range(TILES) if do_c else []):
                S, t0 = divmod(t, SUPER)
                if t0 == 0:
                    if S + 3 < N_SUPER:
                        _issue_c(S + 3, "A", _c_bufs)
                    if S + 1 < N_SUPER:
                        _issue_c(S + 1, "B", _c_bufs)
                if t + 2 < TILES:
                    _build_r2(t + 2)
                mA = _c_bufs[S]["A"]
                mB = _c_bufs[S]["B"]
                # psO[d, f] = segment-sum of g[src] (node-major via lhsT=R),
                # split across two PSUM banks (tags reused from stage A).
                psO0 = ps_h.tile([128, 128], f32, tag="psH")
                psO1 = ps_g.tile([128, 128], f32, tag="psG")
                RA, RB = rq2.pop(t)
                mms = ([(mA, t0 * CA2 + k, RA, k) for k in range(CA2)]
                       + [(mB, t0 * CB2 + k, RB, k) for k in range(CB2)])
                banks = [psO0, psO1]
                nb = [sum(1 for i in range(len(mms)) if i % 2 == b)
                      for b in range(2)]
                cnt = [0, 0]
                for i, (buf, c, R, k) in enumerate(mms):
                    b = i % 2
                    nc.tensor.matmul(banks[b][:], lhsT=R[:, k, :],
                                     rhs=buf[:, c, :], start=(cnt[b] == 0),
                                     stop=(cnt[b] == nb[b] - 1))
                    cnt[b] += 1
                # psR[d, f] = H_d @ W2_r^T + b2 (rank-1 bias matmul)
                psR = ps_seg.tile([128, 128], f32, tag="psS0")
                for j in range(NH):
                    nc.tensor.matmul(psR[:], lhsT=HT[:, j, t * TILE:(t + 1) * TILE],
                                     rhs=w2rT[:, j, :],
                                     start=(j == 0), stop=False)
                nc.tensor.matmul(psR[:], lhsT=ones1[:], rhs=b2r[:],
                                 start=False, stop=True)
                # out = (psO0 + psO1) * rdeg(d) + psR (scales fused in ScalarE)
                sc0 = mpool.tile([128, 128], f32, tag="sc")
                nc.scalar.activation(sc0[:], psO0[:],
                                     mybir.ActivationFunctionType.Copy,
                                     scale=rbT[:, t:t + 1])
                sc1 = mpool.tile([128, 128], f32, tag="sc1")
                nc.scalar.activation(sc1[:], psO1[:],
                                     mybir.ActivationFunctionType.Copy,
                                     scale=rbT[:, t:t + 1])
                u = mpool.tile([128, 128], f32, tag="u")
                nc.vector.tensor_tensor(u[:], sc0[:], sc1[:],
                                        mybir.AluOpType.add)
                oN = spool.tile([128, 128], f32, tag="oN")
                nc.vector.tensor_tensor(oN[:], u[:], psR[:],
                                        mybir.AluOpType.add)
                nc.sync.dma_start(
                    outN_d[t * TILE:(t + 1) * TILE, :], oN[:])

    # Align each gather's SWDGE queue with the DMASW sem lane Tile assigned
    # (sem lane L is locked to one queue; use queue = L % num_queues).
    import re as _re
    for bb in nc.main_func.blocks:
        for ins in bb.instructions:
            if isinstance(ins, mybir.InstDMAGatherAnt):
                lane = None
                si = ins.sync_info
                if si is not None:
                    for upd in list(si.on_update):
                        m = _re.match(r"DMASW(\d+)", getattr(upd, "ant_name", None) or "")
                        if m:
                            lane = int(m.group(1))
                if lane is not None:
                    ins.queue_num = lane % 4
    nc.compile()
    return nc


def kernel(x, edge_index, W1_l, b1_l, W1_r, W2_l, b2_l, W2_r):
    import ml_dtypes
    from concourse.bass_utils import run_bass_kernel_spmd

    x = np.asarray(x, np.float32)
    n_nodes, din = x.shape
    dh = W1_l.shape[0]
    dout = W2_l.shape[0]

    meta = _preprocess(x, edge_index, n_nodes)
    l1, l2 = meta["l1"], meta["l2"]

    n_lo = SPLIT16
    n_hi = n_nodes - SPLIT16
    nc = _build_program(din, dh, dout, l1["CA"], l1["CB"], l2["CA"], l2["CB"],
                        n_lo, n_hi)

    bf = ml_dtypes.bfloat16
    xg = x.astype(bf)
    w1lT = np.ascontiguousarray(np.asarray(W1_l, np.float32).T).astype(bf)  # [din, dh]
    w1rT = np.ascontiguousarray(np.asarray(W1_r, np.float32).T).astype(bf)
    # [dh, dout] -> [128, dh//128, dout]
    w2lT = np.ascontiguousarray(np.asarray(W2_l, np.float32).T).reshape(
        dh // 128, 128, dout).transpose(1, 0, 2).astype(bf)
    w2rT = np.ascontiguousarray(np.asarray(W2_r, np.float32).T).reshape(
        dh // 128, 128, dout).transpose(1, 0, 2).astype(bf)
    b1 = np.ascontiguousarray(
        np.asarray(b1_l, np.float32).reshape(dh // 128, 128).T)  # [128, nh]
    b2r = np.asarray(b2_l, np.float32).reshape(1, dout).astype(bf)
    ones1 = np.ones((1, 128), np.float32).astype(bf)
    CMAX = max(l1["CA"], l1["CB"], l2["CA"], l2["CB"])
    iota = np.ascontiguousarray(np.broadcast_to(
        np.arange(128, dtype=np.float32), (128, CMAX, 128))).astype(bf)

    in_maps = []
    for c in range(N_CORES):
        in_maps.append({
            "xg": xg, "xT": meta["xT"][c],
            "idx1": l1["idx"][c], "idx2": l2["idx"][c],
            "ds1": l1["ds"][c].astype(bf), "ds2": l2["ds"][c].astype(bf),
            "rb": meta["rb"][c], "rbT": meta["rbT"][c],
            "w1lT": w1lT, "w1rT": w1rT, "w2lT": w2lT, "w2rT": w2rT,
            "b1": b1, "b2r": b2r, "ones1": ones1, "iota": iota,
        })

    res = run_bass_kernel_spmd(nc, in_maps, list(range(N_CORES)))

    out = np.empty((n_nodes, dout), np.float32)
    core_of, row_of = meta["core_of"], meta["row_of"]
    outNs = np.stack([np.asarray(res.results[c]["outN"], np.float32)
                      for c in range(N_CORES)])  # [8, SHARD, dout]
    out[:, :] = outNs[core_of, row_of, :]
    return out


# revision 53
# speedup vs baseline: 1.0255x; 1.0255x over previous
"""Trainium2 Bass kernel for a 2-layer mean-aggregation GraphSAGE GNN.

Strategy (8 NeuronCores, SPMD):
  - Nodes are assigned to (core, tile, slot) with degree balancing; each core
    owns 49 tiles x 128 slots = 6272 dst nodes and the ~100k edges into them.
  - Layer 1: per edge-chunk (128 edges) dma_gather x[src] rows from HBM.
    One-hot R[e, d] = (iota == dstslot[e]) is built for a whole tile-group in
    ONE batched DVE is_equal (stride-0 broadcast AP on the dst-slot operand);
    S^T = sum_e M[e,f]^T R[e,d] accumulates on TensorE (PSUM); 1/deg is
    applied at the PSUM evict (rb broadcast multiply) -> mean^T.
    H^T = relu(W1_l @ mean^T + W1_r @ x^T + b1) via matmuls + fused ScalarE.
  - g = h @ W2_l^T computed per tile (node-major), written to DRAM and
    AllGather'd across cores (bf16, split lo/hi for overlap).
  - Layer 2 (node-major): psO[d,f] = sum_e R[e,d]^T g[src e] via lhsT=R;
    psR[d,f] = H W2_r^T + 1*b2^T (rank-1 bias matmul).  out = psO*rdeg + psR
    with the per-partition rdeg scale fused into the ScalarE evict.
Host does index-only preprocessing (permutation, edge chunking, 1/deg) and
the final unshard.
"""

import functools
import numpy as np

N_CORES = 8
TILES = 49  # tiles per core
TILE = 128
SHARD = TILES * TILE  # 6272
SUPER = 7  # tiles per supertile (gather-call granularity)
N_SUPER = TILES // SUPER  # 7
LO_SUPERS = 4  # supertiles in the "lo" AllGather split
LO_ROWS = LO_SUPERS * SUPER * TILE  # 3584
HI_ROWS = SHARD - LO_ROWS  # 2688
SPLIT16 = 32768  # int16 index limit for layer-1 x gather


def _ceil_div(a, b):
    return -(-a // b)


def _wrap_idxs(idx_flat):
    """Wrap a flat int16 index list into the [128, n/16] dma_gather layout:
    index i lives at [i%16, i//16], replicated across the 8 groups of 16
    partitions."""
    n = len(idx_flat)
    assert n % 16 == 0
    w = np.asarray(idx_flat, np.int16).reshape(n // 16, 16).T  # [16, n/16]
    return np.tile(w, (8, 1))  # [128, n/16]


def _preprocess(x, edge_index, n_nodes):
    """Index-only host preprocessing: node permutation, per-core edge chunk
    streams for both layers, degree reciprocals.  Returns a dict of
    per-core/shared arrays plus layout metadata."""
    src = np.asarray(edge_index[0], np.int64)
    dst = np.asarray(edge_index[1], np.int64)
    E = src.shape[0]

    deg = np.bincount(dst, minlength=n_nodes).astype(np.int64)
    rdeg = (1.0 / np.maximum(deg, 1)).astype(np.float32)

    # Degree-balanced permutation: sort nodes by degree desc, deal round-robin
    # over the 392 global tiles; node -> (core, tile, slot).
    order = np.argsort(-deg, kind="stable")
    g_tile = np.empty(n_nodes, np.int64)   # global tile of node
    g_slot = np.empty(n_nodes, np.int64)   # slot within tile
    n_gtiles = N_CORES * TILES
    idx = np.arange(n_nodes)
    g_tile[order] = idx % n_gtiles
    g_slot[order] = idx // n_gtiles
    core_of = g_tile // TILES
    tile_of = g_tile % TILES
    row_of = tile_of * TILE + g_slot  # row within core shard [0, SHARD)

    e_core = core_of[dst]
    e_tile = tile_of[dst]
    e_slot = g_slot[dst]

    # Layer-1 groups: by src id vs int16 limit.
    l1_grp = (src >= SPLIT16).astype(np.int64)  # 0 = lo (idx=src), 1 = hi
    l1_idx = np.where(l1_grp == 0, src, src - SPLIT16)

    # Layer-2 groups: by gathered-g row (AllGather split layout).
    s_core = core_of[src]
    s_row = row_of[src]
    l2_grp = (s_row >= LO_ROWS).astype(np.int64)
    l2_idx = np.where(l2_grp == 0, s_core * LO_ROWS + s_row,
                      s_core * HI_ROWS + (s_row - LO_ROWS))

    def build_layer(grp, gidx):
        """Compute per-(core,tile,group) edge lists; fixed chunk budgets CA/CB
        (max over all cores/tiles); build idx/dstslot streams in supertile
        gather-call order."""
        counts = np.zeros((N_CORES, TILES, 2), np.int64)
        np.add.at(counts, (e_core, e_tile, grp), 1)
        CA = int(_ceil_div(counts[:, :, 0].max(), TILE))
        CB = int(_ceil_div(counts[:, :, 1].max(), TILE))
        # bucket edges
        key = (e_core * TILES + e_tile) * 2 + grp
        eorder = np.argsort(key * (2 * E) + gidx, kind="stable")  # sorted by key then src for DMA locality
        sorted_key = key[eorder]
        starts = np.searchsorted(sorted_key, np.arange(N_CORES * TILES * 2))
        ends = np.searchsorted(sorted_key, np.arange(N_CORES * TILES * 2) + 1)

        NCHUNK = TILES * (CA + CB)
        idx_cols_per_chunk = TILE // 16  # 8
        idx_arr = np.zeros((N_CORES, 128, NCHUNK * idx_cols_per_chunk), np.int16)
        ds_arr = np.full((N_CORES, 128, NCHUNK), -1.0, np.float32)

        for c in range(N_CORES):
            flat_idx = np.zeros(NCHUNK * TILE, np.int16)
            gc = 0  # global chunk cursor within core stream
            for S in range(N_SUPER):
                for g in range(2):
                    nch = CA if g == 0 else CB
                    # per-tile sorted edge pools for this (supertile, group)
                    pools = []
                    for t0 in range(SUPER):
                        t = S * SUPER + t0
                        k = ((c * TILES + t) * 2) + g
                        es = eorder[starts[k]:ends[k]]
                        assert len(es) <= nch * TILE
                        pools.append([es, 0])  # (sorted-by-src edges, cursor)
                    # distribute per gather-call window so each SDMA engine
                    # reads a contiguous sorted src range (HBM row locality):
                    # engine of group-rel position P is P % 16.
                    g_nch = SUPER * nch
                    g_base = gc * TILE  # stream position of group start
                    for q0 in range(0, g_nch, 8):
                        w_ch = np.arange(q0, min(q0 + 8, g_nch))
                        P = (w_ch[:, None] * TILE
                             + np.arange(TILE)[None, :]).ravel()
                        tiles_of = (P // TILE) // nch
                        for t0 in np.unique(tiles_of):
                            Q = P[tiles_of == t0]
                            es, cur = pools[t0]
                            take = min(len(Q), len(es) - cur)
                            if take <= 0:
                                continue
                            Qf = Q[:take]
                            Qe = Qf[np.lexsort((Qf, Qf % 16))]
                            sel = es[cur:cur + take]
                            pools[t0][1] = cur + take
                            ap = g_base + Qe  # absolute stream positions
                            flat_idx[ap] = gidx[sel].astype(np.int16)
                            ds_arr[c, ap % 128, ap // 128] = e_slot[sel]
                    gc += g_nch
            idx_arr[c] = _wrap_idxs(flat_idx)
        return dict(CA=CA, CB=CB, idx=idx_arr, ds=ds_arr)

    l1 = build_layer(l1_grp, l1_idx)
    l2 = build_layer(l2_grp, l2_idx)

    # Per-core x^T in slot order (zeros for pad slots) + 1/deg layouts.
    import ml_dtypes
    din = x.shape[1]
    xT = np.zeros((N_CORES, din, SHARD), np.float32)
    xT[core_of, :, row_of] = np.asarray(x, np.float32)
    xT_bf = xT.astype(ml_dtypes.bfloat16)
    rbn = np.ones((N_CORES, SHARD), np.float32)
    rbn[core_of, row_of] = rdeg
    rb = np.ascontiguousarray(
        np.broadcast_to(rbn[:, None, :], (N_CORES, 128, SHARD))
    ).astype(ml_dtypes.bfloat16)  # [C, 128, SHARD] col-bcast for L1 evict
    rbT = np.ascontiguousarray(
        rbn.reshape(N_CORES, TILES, TILE).transpose(0, 2, 1)
    ).astype(np.float32)  # [C, 128(slot), TILES] per-partition for L2 evict

    meta = dict(l1=l1, l2=l2, xT=xT_bf, rb=rb, rbT=rbT,
                core_of=core_of, row_of=row_of)
    return meta


@functools.lru_cache(maxsize=2)
def _build_program(din, dh, dout, CA1, CB1, CA2, CB2, n_lo, n_hi,
                   do_cc=True, do_c=True, shared_g=True):
    """Build the SPMD Bass/Tile program.  All shapes static."""
    import concourse.bacc as bacc
    import concourse.mybir as mybir
    import concourse.tile as tile
    from concourse.library_config import mlp

    bf16 = mybir.dt.bfloat16
    f32 = mybir.dt.float32
    i16 = mybir.dt.int16

    NC1 = TILES * (CA1 + CB1)
    NC2 = TILES * (CA2 + CB2)
    W1 = NC1 * 8  # idx cols (TILE/16 per chunk)
    W2 = NC2 * 8
    CMAX = max(CA1, CB1, CA2, CB2)

    nc = bacc.Bacc("TRN2", target_bir_lowering=False, debug=False,
                   num_devices=N_CORES, num_swdge_queues=4)

    # ---- I/O tensors ----
    xg = nc.dram_tensor("xg", [n_lo + n_hi, din], bf16, kind="ExternalInput")
    xT_d = nc.dram_tensor("xT", [din, SHARD], bf16, kind="ExternalInput")
    idx1_d = nc.dram_tensor("idx1", [128, W1], i16, kind="ExternalInput")
    idx2_d = nc.dram_tensor("idx2", [128, W2], i16, kind="ExternalInput")
    ds1_d = nc.dram_tensor("ds1", [128, NC1], bf16, kind="ExternalInput")
    ds2_d = nc.dram_tensor("ds2", [128, NC2], bf16, kind="ExternalInput")
    rb_d = nc.dram_tensor("rb", [128, SHARD], bf16, kind="ExternalInput")
    rbT_d = nc.dram_tensor("rbT", [128, TILES], f32, kind="ExternalInput")
    w1lT_d = nc.dram_tensor("w1lT", [din, dh], bf16, kind="ExternalInput")
    w1rT_d = nc.dram_tensor("w1rT", [din, dh], bf16, kind="ExternalInput")
    w2lT_d = nc.dram_tensor("w2lT", [128, dh // 128, dout], bf16, kind="ExternalInput")
    w2rT_d = nc.dram_tensor("w2rT", [128, dh // 128, dout], bf16, kind="ExternalInput")
    b1_d = nc.dram_tensor("b1", [128, dh // 128], f32, kind="ExternalInput")
    b2r_d = nc.dram_tensor("b2r", [1, dout], bf16, kind="ExternalInput")
    ones_d = nc.dram_tensor("ones1", [1, 128], bf16, kind="ExternalInput")
    iota_d = nc.dram_tensor("iota", [128, CMAX, 128], bf16, kind="ExternalInput")
    outN_d = nc.dram_tensor("outN", [SHARD, dout], f32, kind="ExternalOutput")

    # internal DRAM
    gl_lo = nc.dram_tensor("gl_lo", [LO_ROWS, dout], bf16)
    gl_hi = nc.dram_tensor("gl_hi", [HI_ROWS, dout], bf16)
    _aspace = "Shared" if shared_g else None
    gf_lo = nc.dram_tensor("gf_lo", [N_CORES * LO_ROWS, dout], bf16,
                           addr_space=_aspace)
    gf_hi = nc.dram_tensor("gf_hi", [N_CORES * HI_ROWS, dout], bf16,
                           addr_space=_aspace)

    NH = dh // 128  # h halves (2)

    with tile.TileContext(nc) as tc:
        with (
            tc.tile_pool(name="per", bufs=1) as per,       # persistent SBUF
            tc.tile_pool(name="gath", bufs=3) as gpool,    # gather buffers
            tc.tile_pool(name="rt", bufs=3) as rpool,      # one-hot R tiles
            tc.tile_pool(name="mt", bufs=2) as mpool,      # meanT / evict tiles
            tc.tile_pool(name="stg", bufs=3) as spool,     # staging for DRAM writes
            tc.tile_pool(name="ps_seg", bufs=2, space="PSUM") as ps_seg,
            tc.tile_pool(name="ps_h", bufs=2, space="PSUM") as ps_h,
            tc.tile_pool(name="ps_g", bufs=2, space="PSUM") as ps_g,
        ):
            # ---- persistent loads ----
            xT = per.tile([din, SHARD], bf16)
            idx = per.tile([128, max(W1, W2)], i16)  # idx1, then idx2
            ds1 = per.tile([128, NC1], bf16)
            ds2 = per.tile([128, NC2], bf16)
            rb = per.tile([128, SHARD], bf16)
            rbT = per.tile([128, TILES], f32)
            w1lT = per.tile([din, dh], bf16)
            w1rT = per.tile([din, dh], bf16)
            w2lT = per.tile([128, NH, dout], bf16)
            w2rT = per.tile([128, NH, dout], bf16)
            b1 = per.tile([128, NH], f32)
            b2r = per.tile([1, dout], bf16)
            ones1 = per.tile([1, 128], bf16)
            iota = per.tile([128, CMAX, 128], bf16)
            HT = per.tile([128, NH, SHARD], bf16)

            for t_sb, t_dr in [(xT, xT_d), (ds1, ds1_d), (ds2, ds2_d),
                               (rb, rb_d),
                               (rbT, rbT_d), (w1lT, w1lT_d), (w1rT, w1rT_d),
                               (w2lT, w2lT_d), (w2rT, w2rT_d), (b1, b1_d),
                               (b2r, b2r_d), (ones1, ones_d), (iota, iota_d)]:
                nc.sync.dma_start(t_sb[:], t_dr[:])
            nc.sync.dma_start(idx[:, 0:W1], idx1_d[:])

            nc.gpsimd.load_library(mlp)

            xg_lo = xg[0:n_lo, :]
            xg_hi = xg[n_lo:n_lo + n_hi, :]

            # ================= Stage A: layer 1 + H + g =================
            a_bufs = {}

            def _issue_a(S):
                mA = gpool.tile([128, SUPER * CA1, din], bf16, tag="mA")
                mB = gpool.tile([128, SUPER * CB1, din], bf16, tag="mB")
                a_bufs[S] = (mA, mB)
                ca_cols = SUPER * CA1 * 8
                cb_cols = SUPER * CB1 * 8
                col0 = S * (ca_cols + cb_cols)
                for buf, nch, src_ap, c0 in [(mA, SUPER * CA1, xg_lo, col0),
                                             (mB, SUPER * CB1, xg_hi, col0 + ca_cols)]:
                    for q0 in range(0, nch, 8):
                        n = min(8, nch - q0)
                        nc.gpsimd.dma_gather(
                            buf[:, q0:q0 + n, :], src_ap,
                            idx[:, c0 + q0 * 8:c0 + (q0 + n) * 8],
                            n * TILE, n * TILE, din)

            rq1 = {}

            def _build_r1(t):
                S, t0 = divmod(t, SUPER)
                gc0 = S * SUPER * (CA1 + CB1)
                rs = []
                for g, (CC, base) in enumerate(
                        [(CA1, gc0), (CB1, gc0 + SUPER * CA1)]):
                    gcs = base + t0 * CC
                    R = rpool.tile([128, CC, 128], bf16,
                                   tag=("RA" if g == 0 else "RB"))
                    nc.vector.tensor_tensor(
                        R[:], iota[:, 0:CC, :],
                        ds1[:, gcs:gcs + CC].broadcast_to([128, CC, 128]),
                        mybir.AluOpType.is_equal)
                    rs.append(R)
                rq1[t] = rs

            import concourse.mybir as _mb

            def _issue_c(S, which, bufs_by_S):
                ca_cols = SUPER * CA2 * 8
                cb_cols = SUPER * CB2 * 8
                col0 = S * (ca_cols + cb_cols)
                if which == "A":
                    mA2 = gpool.tile([128, SUPER * CA2, dout], bf16, tag="mA")
                    bufs_by_S.setdefault(S, {})["A"] = mA2
                    nch, src_ap, c0, buf = SUPER * CA2, gf_lo[:], col0, mA2
                else:
                    mB2 = gpool.tile([128, SUPER * CB2, dout], bf16, tag="mB")
                    bufs_by_S.setdefault(S, {})["B"] = mB2
                    nch, src_ap, c0, buf = (SUPER * CB2, gf_hi[:],
                                            col0 + ca_cols, mB2)
                for q0 in range(0, nch, 8):
                    n = min(8, nch - q0)
                    nc.gpsimd.dma_gather(
                        buf[:, q0:q0 + n, :], src_ap,
                        idx[:, c0 + q0 * 8:c0 + (q0 + n) * 8],
                        n * TILE, n * TILE, dout)

            _c_bufs = {}
            _issue_a(0)
            _issue_a(1)
            _build_r1(0)
            _build_r1(1)
            for t in range(TILES):
                S, t0 = divmod(t, SUPER)
                if t0 == 0 and S + 2 < N_SUPER:
                    _issue_a(S + 2)
                if t0 == 0 and S == N_SUPER - 3 and do_cc:
                    # every stage-A gather is already issued (depth-2
                    # prefetch), so AG-lo here blocks nothing and its
                    # transfer hides behind the last three supertiles.
                    nc.gpsimd.collective_compute(
                        "AllGather", _mb.AluOpType.bypass,
                        replica_groups=[list(range(N_CORES))],
                        ins=[gl_lo.ap().opt()], outs=[gf_lo.ap().opt()])
                if t + 2 < TILES:
                    _build_r1(t + 2)
                mA, mB = a_bufs[S]
                # segment-sum split over two PSUM banks so consecutive
                # accumulating matmuls overlap (same-bank chains serialize).
                psS0 = ps_seg.tile([128, 128], f32, tag="psS0")
                psS1 = ps_seg.tile([128, 128], f32, tag="psS1")
                RA, RB = rq1.pop(t)
                mms = ([(mA, t0 * CA1 + k, RA, k) for k in range(CA1)]
                       + [(mB, t0 * CB1 + k, RB, k) for k in range(CB1)])
                banks = [psS0, psS1]
                nb = [sum(1 for i in range(len(mms)) if i % 2 == b)
                      for b in range(2)]
                cnt = [0, 0]
                for i, (buf, c, R, k) in enumerate(mms):
                    b = i % 2
                    nc.tensor.matmul(banks[b][:], lhsT=buf[:, c, :],
                                     rhs=R[:, k, :], start=(cnt[b] == 0),
                                     stop=(cnt[b] == nb[b] - 1))
                    cnt[b] += 1
                # meanT = (psS0 + psS1) * rdeg  (ACT evicts bank1, DVE fuses)
                s1 = mpool.tile([128, 128], f32, tag="s1")
                nc.scalar.activation(s1[:], psS1[:],
                                     mybir.ActivationFunctionType.Copy)
                ssum = mpool.tile([128, 128], f32, tag="ssum")
                nc.vector.tensor_tensor(ssum[:], psS0[:], s1[:],
                                        mybir.AluOpType.add)
                meanT = mpool.tile([128, 128], bf16, tag="meanT")
                nc.vector.tensor_tensor(
                    meanT[:], ssum[:], rb[:, t * TILE:(t + 1) * TILE],
                    mybir.AluOpType.mult)
                # H^T halves
                for j in range(NH):
                    psH = ps_h.tile([128, 128], f32, tag="psH")
                    nc.tensor.matmul(psH[:], lhsT=w1lT[:, j * 128:(j + 1) * 128],
                                     rhs=meanT[:], start=True, stop=False)
                    nc.tensor.matmul(psH[:], lhsT=w1rT[:, j * 128:(j + 1) * 128],
                                     rhs=xT[:, t * TILE:(t + 1) * TILE],
                                     start=False, stop=True)
                    nc.scalar.activation(HT[:, j, t * TILE:(t + 1) * TILE], psH[:],
                                         mybir.ActivationFunctionType.Relu,
                                         bias=b1[:, j:j + 1])
                # g tile (node-major)
                psG = ps_g.tile([128, 128], f32, tag="psG")
                for j in range(NH):
                    nc.tensor.matmul(psG[:], lhsT=HT[:, j, t * TILE:(t + 1) * TILE],
                                     rhs=w2lT[:, j, :], start=(j == 0),
                                     stop=(j == NH - 1))
                gT = spool.tile([128, dout], bf16, tag="gT")
                nc.scalar.activation(gT[:], psG[:],
                                     mybir.ActivationFunctionType.Copy)
                row = t * TILE
                if row < LO_ROWS:
                    dst = gl_lo[row:row + TILE, :]
                else:
                    dst = gl_hi[row - LO_ROWS:row - LO_ROWS + TILE, :]
                nc.sync.dma_start(dst, gT[:])

            # idx buffer is free of layer-1 readers once stage-A gathers are
            # issued; load the layer-2 index stream (overlaps the AllGather).
            nc.sync.dma_start(idx[:, 0:W2], idx2_d[:])
            if do_c:
                _issue_c(0, "A", _c_bufs)
                _issue_c(1, "A", _c_bufs)

            # ================= Stage C: layer 2 (node-major) =================
            rq2 = {}

            def _build_r2(t):
                S, t0 = divmod(t, SUPER)
                gc0 = S * SUPER * (CA2 + CB2)
                rs = []
                for g, (CC, base) in enumerate(
                        [(CA2, gc0), (CB2, gc0 + SUPER * CA2)]):
                    gcs = base + t0 * CC
                    R = rpool.tile([128, CC, 128], bf16,
                                   tag=("RA2" if g == 0 else "RB2"))
                    nc.vector.tensor_tensor(
                        R[:], iota[:, 0:CC, :],
                        ds2[:, gcs:gcs + CC].broadcast_to([128, CC, 128]),
                        mybir.AluOpType.is_equal)
                    rs.append(R)
                rq2[t] = rs

            if do_cc:
                nc.gpsimd.collective_compute(
                    "AllGather", _mb.AluOpType.bypass,
                    replica_groups=[list(range(N_CORES))],
                    ins=[gl_hi.ap().opt()], outs=[gf_hi.ap().opt()])
            if do_c:
                _issue_c(2, "A", _c_bufs)
                _issue_c(0, "B", _c_bufs)
                _build_r2(0)
                _build_r2(1)
            for t in (range(TILES) if do_c else []):
                S, t0 = divmod(t, SUPER)
                if t0 == 0:
                    if S + 3 < N_SUPER:
                        _issue_c(S + 3, "A", _c_bufs)
                    if S + 1 < N_SUPER:
                        _issue_c(S + 1, "B", _c_bufs)
                if t + 2 < TILES:
                    _build_r2(t + 2)
                mA = _c_bufs[S]["A"]
                mB = _c_bufs[S]["B"]
                # psO[d, f] = segment-sum of g[src] (node-major via lhsT=R),
                # split across two PSUM banks (tags reused from stage A).
                psO0 = ps_h.tile([128, 128], f32, tag="psH")
                psO1 = ps_g.tile([128, 128], f32, tag="psG")
                RA, RB = rq2.pop(t)
                mms = ([(mA, t0 * CA2 + k, RA, k) for k in range(CA2)]
                       + [(mB, t0 * CB2 + k, RB, k) for k in range(CB2)])
                banks = [psO0, psO1]
                nb = [sum(1 for i in range(len(mms)) if i % 2 == b)
                      for b in range(2)]
                cnt = [0, 0]
                for i, (buf, c, R, k) in enumerate(mms):
                    b = i % 2
                    nc.tensor.matmul(banks[b][:], lhsT=R[:, k, :],
                                     rhs=buf[:, c, :], start=(cnt[b] == 0),
                                     stop=(cnt[b] == nb[b] - 1))
                    cnt[b] += 1
                # psR[d, f] = H_d @ W2_r^T + b2 (rank-1 bias matmul)
                psR = ps_seg.tile([128, 128], f32, tag="psS0")
                for j in range(NH):
                    nc.tensor.matmul(psR[:], lhsT=HT[:, j, t * TILE:(t + 1) * TILE],
                                     rhs=w2rT[:, j, :],
                                     start=(j == 0), stop=False)
                nc.tensor.matmul(psR[:], lhsT=ones1[:], rhs=b2r[:],
                                 start=False, stop=True)
                # out = (psO0 + psO1) * rdeg(d) + psR (scales fused in ScalarE)
                sc0 = mpool.tile([128, 128], f32, tag="sc")
                nc.scalar.activation(sc0[:], psO0[:],
                                     mybir.ActivationFunctionType.Copy,
                                     scale=rbT[:, t:t + 1])
                sc1 = mpool.tile([128, 128], f32, tag="sc1")
                nc.scalar.activation(sc1[:], psO1[:],
                                     mybir.ActivationFunctionType.Copy,
                                     scale=rbT[:, t:t + 1])
                u = mpool.tile([128, 128], f32, tag="u")
                nc.vector.tensor_tensor(u[:], sc0[:], sc1[:],
                                        mybir.AluOpType.add)
                oN = spool.tile([128, 128], f32, tag="oN")
                nc.vector.tensor_tensor(oN[:], u[:], psR[:],
                                        mybir.AluOpType.add)
                nc.sync.dma_start(
                    outN_d[t * TILE:(t + 1) * TILE, :], oN[:])

    # Align each gather's SWDGE queue with the DMASW sem lane Tile assigned
    # (sem lane L is locked to one queue; use queue = L % num_queues).
    import re as _re
    for bb in nc.main_func.blocks:
        for ins in bb.instructions:
            if isinstance(ins, mybir.InstDMAGatherAnt):
                lane = None
                si = ins.sync_info
                if si is not None:
                    for upd in list(si.on_update):
                        m = _re.match(r"DMASW(\d+)", getattr(upd, "ant_name", None) or "")
                        if m:
                            lane = int(m.group(1))
                if lane is not None:
                    ins.queue_num = lane % 4
    nc.compile()
    return nc


def kernel(x, edge_index, W1_l, b1_l, W1_r, W2_l, b2_l, W2_r):
    import ml_dtypes
    from concourse.bass_utils import run_bass_kernel_spmd

    x = np.asarray(x, np.float32)
    n_nodes, din = x.shape
    dh = W1_l.shape[0]
    dout = W2_l.shape[0]

    meta = _preprocess(x, edge_index, n_nodes)
    l1, l2 = meta["l1"], meta["l2"]

    n_lo = SPLIT16
    n_hi = n_nodes - SPLIT16
    nc = _build_program(din, dh, dout, l1["CA"], l1["CB"], l2["CA"], l2["CB"],
                        n_lo, n_hi)

    bf = ml_dtypes.bfloat16
    xg = x.astype(bf)
    w1lT = np.ascontiguousarray(np.asarray(W1_l, np.float32).T).astype(bf)  # [din, dh]
    w1rT = np.ascontiguousarray(np.asarray(W1_r, np.float32).T).astype(bf)
    # [dh, dout] -> [128, dh//128, dout]
    w2lT = np.ascontiguousarray(np.asarray(W2_l, np.float32).T).reshape(
        dh // 128, 128, dout).transpose(1, 0, 2).astype(bf)
    w2rT = np.ascontiguousarray(np.asarray(W2_r, np.float32).T).reshape(
        dh // 128, 128, dout).transpose(1, 0, 2).astype(bf)
    b1 = np.ascontiguousarray(
        np.asarray(b1_l, np.float32).reshape(dh // 128, 128).T)  # [128, nh]
    b2r = np.asarray(b2_l, np.float32).reshape(1, dout).astype(bf)
    ones1 = np.ones((1, 128), np.float32).astype(bf)
    CMAX = max(l1["CA"], l1["CB"], l2["CA"], l2["CB"])
    iota = np.ascontiguousarray(np.broadcast_to(
        np.arange(128, dtype=np.float32), (128, CMAX, 128))).astype(bf)

    in_maps = []
    for c in range(N_CORES):
        in_maps.append({
            "xg": xg, "xT": meta["xT"][c],
            "idx1": l1["idx"][c], "idx2": l2["idx"][c],
            "ds1": l1["ds"][c].astype(bf), "ds2": l2["ds"][c].astype(bf),
            "rb": meta["rb"][c], "rbT": meta["rbT"][c],
            "w1lT": w1lT, "w1rT": w1rT, "w2lT": w2lT, "w2rT": w2rT,
            "b1": b1, "b2r": b2r, "ones1": ones1, "iota": iota,
        })

    res = run_bass_kernel_spmd(nc, in_maps, list(range(N_CORES)))

    out = np.empty((n_nodes, dout), np.float32)
    core_of, row_of = meta["core_of"], meta["row_of"]
    outNs = np.stack([np.asarray(res.results[c]["outN"], np.float32)
                      for c in range(N_CORES)])  # [8, SHARD, dout]
    out[:, :] = outNs[core_of, row_of, :]
    return out


# revision 54
# speedup vs baseline: 1.0810x; 1.0541x over previous
"""Trainium2 Bass kernel for a 2-layer mean-aggregation GraphSAGE GNN.

Strategy (8 NeuronCores, SPMD):
  - Nodes are assigned to (core, tile, slot) with degree balancing; each core
    owns 49 tiles x 128 slots = 6272 dst nodes and the ~100k edges into them.
  - Layer 1: per edge-chunk (128 edges) dma_gather x[src] rows from HBM.
    One-hot R[e, d] = (iota == dstslot[e]) is built for a whole tile-group in
    ONE batched DVE is_equal (stride-0 broadcast AP on the dst-slot operand);
    S^T = sum_e M[e,f]^T R[e,d] accumulates on TensorE (PSUM); 1/deg is
    applied at the PSUM evict (rb broadcast multiply) -> mean^T.
    H^T = relu(W1_l @ mean^T + W1_r @ x^T + b1) via matmuls + fused ScalarE.
  - g = h @ W2_l^T computed per tile (node-major), written to DRAM and
    AllGather'd across cores (bf16, split lo/hi for overlap).
  - Layer 2 (node-major): psO[d,f] = sum_e R[e,d]^T g[src e] via lhsT=R;
    psR[d,f] = H W2_r^T + 1*b2^T (rank-1 bias matmul).  out = psO*rdeg + psR
    with the per-partition rdeg scale fused into the ScalarE evict.
Host does index-only preprocessing (permutation, edge chunking, 1/deg) and
the final unshard.
"""

import functools
import numpy as np

N_CORES = 8
TILES = 49  # tiles per core
TILE = 128
SHARD = TILES * TILE  # 6272
SUPER = 7  # tiles per supertile (gather-call granularity)
N_SUPER = TILES // SUPER  # 7
LO_SUPERS = 4  # supertiles in the "lo" AllGather split
LO_ROWS = LO_SUPERS * SUPER * TILE  # 3584
HI_ROWS = SHARD - LO_ROWS  # 2688
SPLIT16 = 32768  # int16 index limit for layer-1 x gather


def _ceil_div(a, b):
    return -(-a // b)


def _wrap_idxs(idx_flat):
    """Wrap a flat int16 index list into the [128, n/16] dma_gather layout:
    index i lives at [i%16, i//16], replicated across the 8 groups of 16
    partitions."""
    n = len(idx_flat)
    assert n % 16 == 0
    w = np.asarray(idx_flat, np.int16).reshape(n // 16, 16).T  # [16, n/16]
    return np.tile(w, (8, 1))  # [128, n/16]


def _preprocess(x, edge_index, n_nodes):
    """Index-only host preprocessing: node permutation, per-core edge chunk
    streams for both layers, degree reciprocals.  Returns a dict of
    per-core/shared arrays plus layout metadata."""
    src = np.asarray(edge_index[0], np.int64)
    dst = np.asarray(edge_index[1], np.int64)
    E = src.shape[0]

    deg = np.bincount(dst, minlength=n_nodes).astype(np.int64)
    rdeg = (1.0 / np.maximum(deg, 1)).astype(np.float32)

    # Degree-balanced permutation: sort nodes by degree desc, deal round-robin
    # over the 392 global tiles; node -> (core, tile, slot).
    order = np.argsort(-deg, kind="stable")
    g_tile = np.empty(n_nodes, np.int64)   # global tile of node
    g_slot = np.empty(n_nodes, np.int64)   # slot within tile
    n_gtiles = N_CORES * TILES
    idx = np.arange(n_nodes)
    g_tile[order] = idx % n_gtiles
    g_slot[order] = idx // n_gtiles
    core_of = g_tile // TILES
    tile_of = g_tile % TILES
    row_of = tile_of * TILE + g_slot  # row within core shard [0, SHARD)

    e_core = core_of[dst]
    e_tile = tile_of[dst]
    e_slot = g_slot[dst]

    # Layer-1 groups: by src id vs int16 limit.
    l1_grp = (src >= SPLIT16).astype(np.int64)  # 0 = lo (idx=src), 1 = hi
    l1_idx = np.where(l1_grp == 0, src, src - SPLIT16)

    # Layer-2 groups: by gathered-g row (AllGather split layout).
    s_core = core_of[src]
    s_row = row_of[src]
    l2_grp = (s_row >= LO_ROWS).astype(np.int64)
    l2_idx = np.where(l2_grp == 0, s_core * LO_ROWS + s_row,
                      s_core * HI_ROWS + (s_row - LO_ROWS))

    def build_layer(grp, gidx):
        """Compute per-(core,tile,group) edge lists; fixed chunk budgets CA/CB
        (max over all cores/tiles); build idx/dstslot streams in supertile
        gather-call order."""
        counts = np.zeros((N_CORES, TILES, 2), np.int64)
        np.add.at(counts, (e_core, e_tile, grp), 1)
        CA = int(_ceil_div(counts[:, :, 0].max(), TILE))
        CB = int(_ceil_div(counts[:, :, 1].max(), TILE))
        # bucket edges
        key = (e_core * TILES + e_tile) * 2 + grp
        eorder = np.argsort(key * (2 * E) + gidx, kind="stable")  # sorted by key then src for DMA locality
        sorted_key = key[eorder]
        starts = np.searchsorted(sorted_key, np.arange(N_CORES * TILES * 2))
        ends = np.searchsorted(sorted_key, np.arange(N_CORES * TILES * 2) + 1)

        NCHUNK = TILES * (CA + CB)
        idx_cols_per_chunk = TILE // 16  # 8
        idx_arr = np.zeros((N_CORES, 128, NCHUNK * idx_cols_per_chunk), np.int16)
        ds_arr = np.full((N_CORES, 128, NCHUNK), -1.0, np.float32)

        for c in range(N_CORES):
            flat_idx = np.zeros(NCHUNK * TILE, np.int16)
            gc = 0  # global chunk cursor within core stream
            for S in range(N_SUPER):
                for g in range(2):
                    nch = CA if g == 0 else CB
                    # per-tile sorted edge pools for this (supertile, group)
                    pools = []
                    for t0 in range(SUPER):
                        t = S * SUPER + t0
                        k = ((c * TILES + t) * 2) + g
                        es = eorder[starts[k]:ends[k]]
                        assert len(es) <= nch * TILE
                        pools.append([es, 0])  # (sorted-by-src edges, cursor)
                    # distribute per gather-call window so each SDMA engine
                    # reads a contiguous sorted src range (HBM row locality):
                    # engine of group-rel position P is P % 16.
                    g_nch = SUPER * nch
                    g_base = gc * TILE  # stream position of group start
                    for q0 in range(0, g_nch, 8):
                        w_ch = np.arange(q0, min(q0 + 8, g_nch))
                        P = (w_ch[:, None] * TILE
                             + np.arange(TILE)[None, :]).ravel()
                        tiles_of = (P // TILE) // nch
                        for t0 in np.unique(tiles_of):
                            Q = P[tiles_of == t0]
                            es, cur = pools[t0]
                            take = min(len(Q), len(es) - cur)
                            if take <= 0:
                                continue
                            Qf = Q[:take]
                            Qe = Qf[np.lexsort((Qf, Qf % 16))]
                            sel = es[cur:cur + take]
                            pools[t0][1] = cur + take
                            ap = g_base + Qe  # absolute stream positions
                            flat_idx[ap] = gidx[sel].astype(np.int16)
                            ds_arr[c, ap % 128, ap // 128] = e_slot[sel]
                    gc += g_nch
            idx_arr[c] = _wrap_idxs(flat_idx)
        return dict(CA=CA, CB=CB, idx=idx_arr, ds=ds_arr)

    l1 = build_layer(l1_grp, l1_idx)
    l2 = build_layer(l2_grp, l2_idx)

    # Per-core x^T in slot order (zeros for pad slots) + 1/deg layouts.
    import ml_dtypes
    din = x.shape[1]
    xT = np.zeros((N_CORES, din, SHARD), np.float32)
    xT[core_of, :, row_of] = np.asarray(x, np.float32)
    xT_bf = xT.astype(ml_dtypes.bfloat16)
    rbn = np.ones((N_CORES, SHARD), np.float32)
    rbn[core_of, row_of] = rdeg
    rb = np.ascontiguousarray(
        np.broadcast_to(rbn[:, None, :], (N_CORES, 128, SHARD))
    ).astype(ml_dtypes.bfloat16)  # [C, 128, SHARD] col-bcast for L1 evict
    rbT = np.ascontiguousarray(
        rbn.reshape(N_CORES, TILES, TILE).transpose(0, 2, 1)
    ).astype(np.float32)  # [C, 128(slot), TILES] per-partition for L2 evict

    meta = dict(l1=l1, l2=l2, xT=xT_bf, rb=rb, rbT=rbT,
                core_of=core_of, row_of=row_of)
    return meta


@functools.lru_cache(maxsize=2)
def _build_program(din, dh, dout, CA1, CB1, CA2, CB2, n_lo, n_hi,
                   do_cc=True, do_c=True, shared_g=True):
    """Build the SPMD Bass/Tile program.  All shapes static."""
    import concourse.bacc as bacc
    import concourse.mybir as mybir
    import concourse.tile as tile
    from concourse.library_config import mlp

    bf16 = mybir.dt.bfloat16
    f32 = mybir.dt.float32
    i16 = mybir.dt.int16

    NC1 = TILES * (CA1 + CB1)
    NC2 = TILES * (CA2 + CB2)
    W1 = NC1 * 8  # idx cols (TILE/16 per chunk)
    W2 = NC2 * 8
    CMAX = max(CA1, CB1, CA2, CB2)

    nc = bacc.Bacc("TRN2", target_bir_lowering=False, debug=False,
                   num_devices=N_CORES, num_swdge_queues=4)

    # ---- I/O tensors ----
    xg = nc.dram_tensor("xg", [n_lo + n_hi, din], bf16, kind="ExternalInput")
    xT_d = nc.dram_tensor("xT", [din, SHARD], bf16, kind="ExternalInput")
    idx1_d = nc.dram_tensor("idx1", [128, W1], i16, kind="ExternalInput")
    idx2_d = nc.dram_tensor("idx2", [128, W2], i16, kind="ExternalInput")
    ds1_d = nc.dram_tensor("ds1", [128, NC1], bf16, kind="ExternalInput")
    ds2_d = nc.dram_tensor("ds2", [128, NC2], bf16, kind="ExternalInput")
    rb_d = nc.dram_tensor("rb", [128, SHARD], bf16, kind="ExternalInput")
    rbT_d = nc.dram_tensor("rbT", [128, TILES], f32, kind="ExternalInput")
    w1lT_d = nc.dram_tensor("w1lT", [din, dh], bf16, kind="ExternalInput")
    w1rT_d = nc.dram_tensor("w1rT", [din, dh], bf16, kind="ExternalInput")
    w2lT_d = nc.dram_tensor("w2lT", [128, dh // 128, dout], bf16, kind="ExternalInput")
    w2rT_d = nc.dram_tensor("w2rT", [128, dh // 128, dout], bf16, kind="ExternalInput")
    b1_d = nc.dram_tensor("b1", [128, dh // 128], f32, kind="ExternalInput")
    b2r_d = nc.dram_tensor("b2r", [1, dout], bf16, kind="ExternalInput")
    ones_d = nc.dram_tensor("ones1", [1, 128], bf16, kind="ExternalInput")
    iota_d = nc.dram_tensor("iota", [128, CMAX, 128], bf16, kind="ExternalInput")
    outN_d = nc.dram_tensor("outN", [SHARD, dout], f32, kind="ExternalOutput")

    # internal DRAM
    gl_lo = nc.dram_tensor("gl_lo", [LO_ROWS, dout], bf16)
    gl_hi = nc.dram_tensor("gl_hi", [HI_ROWS, dout], bf16)
    _aspace = "Shared" if shared_g else None
    gf_lo = nc.dram_tensor("gf_lo", [N_CORES * LO_ROWS, dout], bf16,
                           addr_space=_aspace)
    gf_hi = nc.dram_tensor("gf_hi", [N_CORES * HI_ROWS, dout], bf16,
                           addr_space=_aspace)

    NH = dh // 128  # h halves (2)

    with tile.TileContext(nc) as tc:
        with (
            tc.tile_pool(name="per", bufs=1) as per,       # persistent SBUF
            tc.tile_pool(name="gath", bufs=3) as gpool,    # gather buffers
            tc.tile_pool(name="rt", bufs=3) as rpool,      # one-hot R tiles
            tc.tile_pool(name="mt", bufs=2) as mpool,      # meanT / evict tiles
            tc.tile_pool(name="stg", bufs=3) as spool,     # staging for DRAM writes
            tc.tile_pool(name="ps_seg", bufs=2, space="PSUM") as ps_seg,
            tc.tile_pool(name="ps_h", bufs=2, space="PSUM") as ps_h,
            tc.tile_pool(name="ps_g", bufs=2, space="PSUM") as ps_g,
        ):
            # ---- persistent loads ----
            xT = per.tile([din, SHARD], bf16)
            idx = per.tile([128, max(W1, W2)], i16)  # idx1, then idx2
            ds1 = per.tile([128, NC1], bf16)
            ds2 = per.tile([128, NC2], bf16)
            rb = per.tile([128, SHARD], bf16)
            rbT = per.tile([128, TILES], f32)
            w1lT = per.tile([din, dh], bf16)
            w1rT = per.tile([din, dh], bf16)
            w2lT = per.tile([128, NH, dout], bf16)
            w2rT = per.tile([128, NH, dout], bf16)
            b1 = per.tile([128, NH], f32)
            b2r = per.tile([1, dout], bf16)
            ones1 = per.tile([1, 128], bf16)
            iota = per.tile([128, CMAX, 128], bf16)
            HT = per.tile([128, NH, SHARD], bf16)

            # idx1 first: the opening gathers wait on it, and the sync FIFO
            # loads in order.  Then the R-build/H inputs, then the rest.
            nc.sync.dma_start(idx[:, 0:W1], idx1_d[:])
            for t_sb, t_dr in [(ds1, ds1_d), (iota, iota_d), (rb, rb_d),
                               (w1lT, w1lT_d), (w1rT, w1rT_d), (b1, b1_d),
                               (xT, xT_d), (w2lT, w2lT_d), (w2rT, w2rT_d),
                               (ds2, ds2_d), (rbT, rbT_d),
                               (b2r, b2r_d), (ones1, ones_d)]:
                nc.sync.dma_start(t_sb[:], t_dr[:])

            nc.gpsimd.load_library(mlp)

            xg_lo = xg[0:n_lo, :]
            xg_hi = xg[n_lo:n_lo + n_hi, :]

            # ================= Stage A: layer 1 + H + g =================
            a_bufs = {}

            def _issue_a(S):
                mA = gpool.tile([128, SUPER * CA1, din], bf16, tag="mA")
                mB = gpool.tile([128, SUPER * CB1, din], bf16, tag="mB")
                a_bufs[S] = (mA, mB)
                ca_cols = SUPER * CA1 * 8
                cb_cols = SUPER * CB1 * 8
                col0 = S * (ca_cols + cb_cols)
                for buf, nch, src_ap, c0 in [(mA, SUPER * CA1, xg_lo, col0),
                                             (mB, SUPER * CB1, xg_hi, col0 + ca_cols)]:
                    for q0 in range(0, nch, 8):
                        n = min(8, nch - q0)
                        nc.gpsimd.dma_gather(
                            buf[:, q0:q0 + n, :], src_ap,
                            idx[:, c0 + q0 * 8:c0 + (q0 + n) * 8],
                            n * TILE, n * TILE, din)

            rq1 = {}

            def _build_r1(t):
                S, t0 = divmod(t, SUPER)
                gc0 = S * SUPER * (CA1 + CB1)
                rs = []
                for g, (CC, base) in enumerate(
                        [(CA1, gc0), (CB1, gc0 + SUPER * CA1)]):
                    gcs = base + t0 * CC
                    R = rpool.tile([128, CC, 128], bf16,
                                   tag=("RA" if g == 0 else "RB"))
                    nc.vector.tensor_tensor(
                        R[:], iota[:, 0:CC, :],
                        ds1[:, gcs:gcs + CC].broadcast_to([128, CC, 128]),
                        mybir.AluOpType.is_equal)
                    rs.append(R)
                rq1[t] = rs

            import concourse.mybir as _mb

            def _issue_c(S, which, bufs_by_S):
                ca_cols = SUPER * CA2 * 8
                cb_cols = SUPER * CB2 * 8
                col0 = S * (ca_cols + cb_cols)
                if which == "A":
                    mA2 = gpool.tile([128, SUPER * CA2, dout], bf16, tag="mA")
                    bufs_by_S.setdefault(S, {})["A"] = mA2
                    nch, src_ap, c0, buf = SUPER * CA2, gf_lo[:], col0, mA2
                else:
                    mB2 = gpool.tile([128, SUPER * CB2, dout], bf16, tag="mB")
                    bufs_by_S.setdefault(S, {})["B"] = mB2
                    nch, src_ap, c0, buf = (SUPER * CB2, gf_hi[:],
                                            col0 + ca_cols, mB2)
                for q0 in range(0, nch, 8):
                    n = min(8, nch - q0)
                    nc.gpsimd.dma_gather(
                        buf[:, q0:q0 + n, :], src_ap,
                        idx[:, c0 + q0 * 8:c0 + (q0 + n) * 8],
                        n * TILE, n * TILE, dout)

            _c_bufs = {}
            _issue_a(0)
            _issue_a(1)
            _build_r1(0)
            _build_r1(1)
            for t in range(TILES):
                S, t0 = divmod(t, SUPER)
                if t0 == 0 and S + 2 < N_SUPER:
                    _issue_a(S + 2)
                if t0 == 0 and S == N_SUPER - 3 and do_cc:
                    # every stage-A gather is already issued (depth-2
                    # prefetch), so AG-lo here blocks nothing and its
                    # transfer hides behind the last three supertiles.
                    nc.gpsimd.collective_compute(
                        "AllGather", _mb.AluOpType.bypass,
                        replica_groups=[list(range(N_CORES))],
                        ins=[gl_lo.ap().opt()], outs=[gf_lo.ap().opt()])
                if t + 2 < TILES:
                    _build_r1(t + 2)
                mA, mB = a_bufs[S]
                # segment-sum split over two PSUM banks so consecutive
                # accumulating matmuls overlap (same-bank chains serialize).
                psS0 = ps_seg.tile([128, 128], f32, tag="psS0")
                psS1 = ps_seg.tile([128, 128], f32, tag="psS1")
                RA, RB = rq1.pop(t)
                mms = ([(mA, t0 * CA1 + k, RA, k) for k in range(CA1)]
                       + [(mB, t0 * CB1 + k, RB, k) for k in range(CB1)])
                banks = [psS0, psS1]
                nb = [sum(1 for i in range(len(mms)) if i % 2 == b)
                      for b in range(2)]
                cnt = [0, 0]
                for i, (buf, c, R, k) in enumerate(mms):
                    b = i % 2
                    nc.tensor.matmul(banks[b][:], lhsT=buf[:, c, :],
                                     rhs=R[:, k, :], start=(cnt[b] == 0),
                                     stop=(cnt[b] == nb[b] - 1))
                    cnt[b] += 1
                # meanT = (psS0 + psS1) * rdeg  (ACT evicts bank1, DVE fuses)
                s1 = mpool.tile([128, 128], f32, tag="s1")
                nc.scalar.activation(s1[:], psS1[:],
                                     mybir.ActivationFunctionType.Copy)
                ssum = mpool.tile([128, 128], f32, tag="ssum")
                nc.vector.tensor_tensor(ssum[:], psS0[:], s1[:],
                                        mybir.AluOpType.add)
                meanT = mpool.tile([128, 128], bf16, tag="meanT")
                nc.vector.tensor_tensor(
                    meanT[:], ssum[:], rb[:, t * TILE:(t + 1) * TILE],
                    mybir.AluOpType.mult)
                # H^T halves
                for j in range(NH):
                    psH = ps_h.tile([128, 128], f32, tag="psH")
                    nc.tensor.matmul(psH[:], lhsT=w1lT[:, j * 128:(j + 1) * 128],
                                     rhs=meanT[:], start=True, stop=False)
                    nc.tensor.matmul(psH[:], lhsT=w1rT[:, j * 128:(j + 1) * 128],
                                     rhs=xT[:, t * TILE:(t + 1) * TILE],
                                     start=False, stop=True)
                    nc.scalar.activation(HT[:, j, t * TILE:(t + 1) * TILE], psH[:],
                                         mybir.ActivationFunctionType.Relu,
                                         bias=b1[:, j:j + 1])
                # g tile (node-major)
                psG = ps_g.tile([128, 128], f32, tag="psG")
                for j in range(NH):
                    nc.tensor.matmul(psG[:], lhsT=HT[:, j, t * TILE:(t + 1) * TILE],
                                     rhs=w2lT[:, j, :], start=(j == 0),
                                     stop=(j == NH - 1))
                gT = spool.tile([128, dout], bf16, tag="gT")
                nc.scalar.activation(gT[:], psG[:],
                                     mybir.ActivationFunctionType.Copy)
                row = t * TILE
                if row < LO_ROWS:
                    dst = gl_lo[row:row + TILE, :]
                else:
                    dst = gl_hi[row - LO_ROWS:row - LO_ROWS + TILE, :]
                nc.sync.dma_start(dst, gT[:])

            # idx buffer is free of layer-1 readers once stage-A gathers are
            # issued; load the layer-2 index stream (overlaps the AllGather).
            nc.sync.dma_start(idx[:, 0:W2], idx2_d[:])
            if do_c:
                _issue_c(0, "A", _c_bufs)
                _issue_c(1, "A", _c_bufs)

            # ================= Stage C: layer 2 (node-major) =================
            rq2 = {}

            def _build_r2(t):
                S, t0 = divmod(t, SUPER)
                gc0 = S * SUPER * (CA2 + CB2)
                rs = []
                for g, (CC, base) in enumerate(
                        [(CA2, gc0), (CB2, gc0 + SUPER * CA2)]):
                    gcs = base + t0 * CC
                    R = rpool.tile([128, CC, 128], bf16,
                                   tag=("RA2" if g == 0 else "RB2"))
                    nc.vector.tensor_tensor(
                        R[:], iota[:, 0:CC, :],
                        ds2[:, gcs:gcs + CC].broadcast_to([128, CC, 128]),
                        mybir.AluOpType.is_equal)
                    rs.append(R)
                rq2[t] = rs

            if do_cc:
                nc.gpsimd.collective_compute(
                    "AllGather", _mb.AluOpType.bypass,
                    replica_groups=[list(range(N_CORES))],
                    ins=[gl_hi.ap().opt()], outs=[gf_hi.ap().opt()])
            if do_c:
                _issue_c(2, "A", _c_bufs)
                _issue_c(0, "B", _c_bufs)
                _build_r2(0)
                _build_r2(1)
            for t in (range(TILES) if do_c else []):
                S, t0 = divmod(t, SUPER)
                if t0 == 0:
                    if S + 3 < N_SUPER:
                        _issue_c(S + 3, "A", _c_bufs)
                    if S + 1 < N_SUPER:
                        _issue_c(S + 1, "B", _c_bufs)
                if t + 2 < TILES:
                    _build_r2(t + 2)
                mA = _c_bufs[S]["A"]
                mB = _c_bufs[S]["B"]
                # psO[d, f] = segment-sum of g[src] (node-major via lhsT=R),
                # split across two PSUM banks (tags reused from stage A).
                psO0 = ps_h.tile([128, 128], f32, tag="psH")
                psO1 = ps_g.tile([128, 128], f32, tag="psG")
                RA, RB = rq2.pop(t)
                mms = ([(mA, t0 * CA2 + k, RA, k) for k in range(CA2)]
                       + [(mB, t0 * CB2 + k, RB, k) for k in range(CB2)])
                banks = [psO0, psO1]
                nb = [sum(1 for i in range(len(mms)) if i % 2 == b)
                      for b in range(2)]
                cnt = [0, 0]
                for i, (buf, c, R, k) in enumerate(mms):
                    b = i % 2
                    nc.tensor.matmul(banks[b][:], lhsT=R[:, k, :],
                                     rhs=buf[:, c, :], start=(cnt[b] == 0),
                                     stop=(cnt[b] == nb[b] - 1))
                    cnt[b] += 1
                # psR[d, f] = H_d @ W2_r^T + b2 (rank-1 bias matmul)
                psR = ps_seg.tile([128, 128], f32, tag="psS0")
                for j in range(NH):
                    nc.tensor.matmul(psR[:], lhsT=HT[:, j, t * TILE:(t + 1) * TILE],
                                     rhs=w2rT[:, j, :],
                                     start=(j == 0), stop=False)
                nc.tensor.matmul(psR[:], lhsT=ones1[:], rhs=b2r[:],
                                 start=False, stop=True)
                # out = (psO0 + psO1) * rdeg(d) + psR (scales fused in ScalarE)
                sc0 = mpool.tile([128, 128], f32, tag="sc")
                nc.scalar.activation(sc0[:], psO0[:],
                                     mybir.ActivationFunctionType.Copy,
                                     scale=rbT[:, t:t + 1])
                sc1 = mpool.tile([128, 128], f32, tag="sc1")
                nc.scalar.activation(sc1[:], psO1[:],
                                     mybir.ActivationFunctionType.Copy,
                                     scale=rbT[:, t:t + 1])
                u = mpool.tile([128, 128], f32, tag="u")
                nc.vector.tensor_tensor(u[:], sc0[:], sc1[:],
                                        mybir.AluOpType.add)
                oN = spool.tile([128, 128], f32, tag="oN")
                nc.vector.tensor_tensor(oN[:], u[:], psR[:],
                                        mybir.AluOpType.add)
                nc.sync.dma_start(
                    outN_d[t * TILE:(t + 1) * TILE, :], oN[:])

    # Align each gather's SWDGE queue with the DMASW sem lane Tile assigned
    # (sem lane L is locked to one queue; use queue = L % num_queues).
    import re as _re
    for bb in nc.main_func.blocks:
        for ins in bb.instructions:
            if isinstance(ins, mybir.InstDMAGatherAnt):
                lane = None
                si = ins.sync_info
                if si is not None:
                    for upd in list(si.on_update):
                        m = _re.match(r"DMASW(\d+)", getattr(upd, "ant_name", None) or "")
                        if m:
                            lane = int(m.group(1))
                if lane is not None:
                    ins.queue_num = lane % 4
    nc.compile()
    return nc


def kernel(x, edge_index, W1_l, b1_l, W1_r, W2_l, b2_l, W2_r):
    import ml_dtypes
    from concourse.bass_utils import run_bass_kernel_spmd

    x = np.asarray(x, np.float32)
    n_nodes, din = x.shape
    dh = W1_l.shape[0]
    dout = W2_l.shape[0]

    meta = _preprocess(x, edge_index, n_nodes)
    l1, l2 = meta["l1"], meta["l2"]

    n_lo = SPLIT16
    n_hi = n_nodes - SPLIT16
    nc = _build_program(din, dh, dout, l1["CA"], l1["CB"], l2["CA"], l2["CB"],
                        n_lo, n_hi)

    bf = ml_dtypes.bfloat16
    xg = x.astype(bf)
    w1lT = np.ascontiguousarray(np.asarray(W1_l, np.float32).T).astype(bf)  # [din, dh]
    w1rT = np.ascontiguousarray(np.asarray(W1_r, np.float32).T).astype(bf)
    # [dh, dout] -> [128, dh//128, dout]
    w2lT = np.ascontiguousarray(np.asarray(W2_l, np.float32).T).reshape(
        dh // 128, 128, dout).transpose(1, 0, 2).astype(bf)
    w2rT = np.ascontiguousarray(np.asarray(W2_r, np.float32).T).reshape(
        dh // 128, 128, dout).transpose(1, 0, 2).astype(bf)
    b1 = np.ascontiguousarray(
        np.asarray(b1_l, np.float32).reshape(dh // 128, 128).T)  # [128, nh]
    b2r = np.asarray(b2_l, np.float32).reshape(1, dout).astype(bf)
    ones1 = np.ones((1, 128), np.float32).astype(bf)
    CMAX = max(l1["CA"], l1["CB"], l2["CA"], l2["CB"])
    iota = np.ascontiguousarray(np.broadcast_to(
        np.arange(128, dtype=np.float32), (128, CMAX, 128))).astype(bf)

    in_maps = []
    for c in range(N_CORES):
        in_maps.append({
            "xg": xg, "xT": meta["xT"][c],
            "idx1": l1["idx"][c], "idx2": l2["idx"][c],
            "ds1": l1["ds"][c].astype(bf), "ds2": l2["ds"][c].astype(bf),
            "rb": meta["rb"][c], "rbT": meta["rbT"][c],
            "w1lT": w1lT, "w1rT": w1rT, "w2lT": w2lT, "w2rT": w2rT,
            "b1": b1, "b2r": b2r, "ones1": ones1, "iota": iota,
        })

    res = run_bass_kernel_spmd(nc, in_maps, list(range(N_CORES)))

    out = np.empty((n_nodes, dout), np.float32)
    core_of, row_of = meta["core_of"], meta["row_of"]
    outNs = np.stack([np.asarray(res.results[c]["outN"], np.float32)
                      for c in range(N_CORES)])  # [8, SHARD, dout]
    out[:, :] = outNs[core_of, row_of, :]
    return out


# revision 56
# speedup vs baseline: 1.1100x; 1.0268x over previous
"""Trainium2 Bass kernel for a 2-layer mean-aggregation GraphSAGE GNN.

Strategy (8 NeuronCores, SPMD):
  - Nodes are assigned to (core, tile, slot) with degree balancing; each core
    owns 49 tiles x 128 slots = 6272 dst nodes and the ~100k edges into them.
  - Layer 1: per edge-chunk (128 edges) dma_gather x[src] rows from HBM.
    One-hot R[e, d] = (iota == dstslot[e]) is built for a whole tile-group in
    ONE batched DVE is_equal (stride-0 broadcast AP on the dst-slot operand);
    S^T = sum_e M[e,f]^T R[e,d] accumulates on TensorE (PSUM); 1/deg is
    applied at the PSUM evict (rb broadcast multiply) -> mean^T.
    H^T = relu(W1_l @ mean^T + W1_r @ x^T + b1) via matmuls + fused ScalarE.
  - g = h @ W2_l^T computed per tile (node-major), written to DRAM and
    AllGather'd across cores (bf16, split lo/hi for overlap).
  - Layer 2 (node-major): psO[d,f] = sum_e R[e,d]^T g[src e] via lhsT=R;
    psR[d,f] = H W2_r^T + 1*b2^T (rank-1 bias matmul).  out = psO*rdeg + psR
    with the per-partition rdeg scale fused into the ScalarE evict.
Host does index-only preprocessing (permutation, edge chunking, 1/deg) and
the final unshard.
"""

import functools
import numpy as np

N_CORES = 8
TILES = 49  # tiles per core
TILE = 128
SHARD = TILES * TILE  # 6272
SUPER = 7  # tiles per supertile (gather-call granularity)
N_SUPER = TILES // SUPER  # 7
LO_SUPERS = 4  # supertiles in the "lo" AllGather split
LO_ROWS = LO_SUPERS * SUPER * TILE  # 3584
HI_ROWS = SHARD - LO_ROWS  # 2688
SPLIT16 = 32768  # int16 index limit for layer-1 x gather


def _ceil_div(a, b):
    return -(-a // b)


def _wrap_idxs(idx_flat):
    """Wrap a flat int16 index list into the [128, n/16] dma_gather layout:
    index i lives at [i%16, i//16], replicated across the 8 groups of 16
    partitions."""
    n = len(idx_flat)
    assert n % 16 == 0
    w = np.asarray(idx_flat, np.int16).reshape(n // 16, 16).T  # [16, n/16]
    return np.tile(w, (8, 1))  # [128, n/16]


def _preprocess(x, edge_index, n_nodes):
    """Index-only host preprocessing: node permutation, per-core edge chunk
    streams for both layers, degree reciprocals.  Returns a dict of
    per-core/shared arrays plus layout metadata."""
    src = np.asarray(edge_index[0], np.int64)
    dst = np.asarray(edge_index[1], np.int64)
    E = src.shape[0]

    deg = np.bincount(dst, minlength=n_nodes).astype(np.int64)
    rdeg = (1.0 / np.maximum(deg, 1)).astype(np.float32)

    # Degree-balanced permutation: sort nodes by degree desc, deal round-robin
    # over the 392 global tiles; node -> (core, tile, slot).
    order = np.argsort(-deg, kind="stable")
    g_tile = np.empty(n_nodes, np.int64)   # global tile of node
    g_slot = np.empty(n_nodes, np.int64)   # slot within tile
    n_gtiles = N_CORES * TILES
    idx = np.arange(n_nodes)
    g_tile[order] = idx % n_gtiles
    g_slot[order] = idx // n_gtiles
    core_of = g_tile // TILES
    tile_of = g_tile % TILES
    row_of = tile_of * TILE + g_slot  # row within core shard [0, SHARD)

    e_core = core_of[dst]
    e_tile = tile_of[dst]
    e_slot = g_slot[dst]

    # Layer-1 groups: by src id vs int16 limit.
    l1_grp = (src >= SPLIT16).astype(np.int64)  # 0 = lo (idx=src), 1 = hi
    l1_idx = np.where(l1_grp == 0, src, src - SPLIT16)

    # Layer-2 groups: by gathered-g row (AllGather split layout).
    s_core = core_of[src]
    s_row = row_of[src]
    l2_grp = (s_row >= LO_ROWS).astype(np.int64)
    l2_idx = np.where(l2_grp == 0, s_core * LO_ROWS + s_row,
                      s_core * HI_ROWS + (s_row - LO_ROWS))

    def build_layer(grp, gidx):
        """Compute per-(core,tile,group) edge lists; fixed chunk budgets CA/CB
        (max over all cores/tiles); build idx/dstslot streams in supertile
        gather-call order."""
        counts = np.zeros((N_CORES, TILES, 2), np.int64)
        np.add.at(counts, (e_core, e_tile, grp), 1)
        CA = int(_ceil_div(counts[:, :, 0].max(), TILE))
        CB = int(_ceil_div(counts[:, :, 1].max(), TILE))
        # bucket edges
        key = (e_core * TILES + e_tile) * 2 + grp
        eorder = np.argsort(key * (2 * E) + gidx, kind="stable")  # sorted by key then src for DMA locality
        sorted_key = key[eorder]
        starts = np.searchsorted(sorted_key, np.arange(N_CORES * TILES * 2))
        ends = np.searchsorted(sorted_key, np.arange(N_CORES * TILES * 2) + 1)

        NCHUNK = TILES * (CA + CB)
        idx_cols_per_chunk = TILE // 16  # 8
        idx_arr = np.zeros((N_CORES, 128, NCHUNK * idx_cols_per_chunk), np.int16)
        ds_arr = np.full((N_CORES, 128, NCHUNK), -1.0, np.float32)

        for c in range(N_CORES):
            flat_idx = np.zeros(NCHUNK * TILE, np.int16)
            gc = 0  # global chunk cursor within core stream
            for S in range(N_SUPER):
                for g in range(2):
                    nch = CA if g == 0 else CB
                    # per-tile sorted edge pools for this (supertile, group)
                    pools = []
                    for t0 in range(SUPER):
                        t = S * SUPER + t0
                        k = ((c * TILES + t) * 2) + g
                        es = eorder[starts[k]:ends[k]]
                        assert len(es) <= nch * TILE
                        pools.append([es, 0])  # (sorted-by-src edges, cursor)
                    # distribute per gather-call window so each SDMA engine
                    # reads a contiguous sorted src range (HBM row locality):
                    # engine of group-rel position P is P % 16.
                    g_nch = SUPER * nch
                    g_base = gc * TILE  # stream position of group start
                    for q0 in range(0, g_nch, 8):
                        w_ch = np.arange(q0, min(q0 + 8, g_nch))
                        P = (w_ch[:, None] * TILE
                             + np.arange(TILE)[None, :]).ravel()
                        tiles_of = (P // TILE) // nch
                        for t0 in np.unique(tiles_of):
                            Q = P[tiles_of == t0]
                            es, cur = pools[t0]
                            take = min(len(Q), len(es) - cur)
                            if take <= 0:
                                continue
                            Qf = Q[:take]
                            Qe = Qf[np.lexsort((Qf, Qf % 16))]
                            sel = es[cur:cur + take]
                            pools[t0][1] = cur + take
                            ap = g_base + Qe  # absolute stream positions
                            flat_idx[ap] = gidx[sel].astype(np.int16)
                            ds_arr[c, ap % 128, ap // 128] = e_slot[sel]
                    gc += g_nch
            idx_arr[c] = _wrap_idxs(flat_idx)
        return dict(CA=CA, CB=CB, idx=idx_arr, ds=ds_arr)

    l1 = build_layer(l1_grp, l1_idx)
    l2 = build_layer(l2_grp, l2_idx)

    # Per-core x^T in slot order (zeros for pad slots) + 1/deg layouts.
    import ml_dtypes
    din = x.shape[1]
    xT = np.zeros((N_CORES, din, SHARD), np.float32)
    xT[core_of, :, row_of] = np.asarray(x, np.float32)
    xT_bf = xT.astype(ml_dtypes.bfloat16)
    rbn = np.ones((N_CORES, SHARD), np.float32)
    rbn[core_of, row_of] = rdeg
    rb = np.ascontiguousarray(
        np.broadcast_to(rbn[:, None, :], (N_CORES, 128, SHARD))
    ).astype(ml_dtypes.bfloat16)  # [C, 128, SHARD] col-bcast for L1 evict
    rbT = np.ascontiguousarray(
        rbn.reshape(N_CORES, TILES, TILE).transpose(0, 2, 1)
    ).astype(np.float32)  # [C, 128(slot), TILES] per-partition for L2 evict

    meta = dict(l1=l1, l2=l2, xT=xT_bf, rb=rb, rbT=rbT,
                core_of=core_of, row_of=row_of)
    return meta


@functools.lru_cache(maxsize=2)
def _build_program(din, dh, dout, CA1, CB1, CA2, CB2, n_lo, n_hi,
                   do_cc=True, do_c=True, shared_g=True):
    """Build the SPMD Bass/Tile program.  All shapes static."""
    import concourse.bacc as bacc
    import concourse.mybir as mybir
    import concourse.tile as tile
    from concourse.library_config import mlp

    bf16 = mybir.dt.bfloat16
    f32 = mybir.dt.float32
    i16 = mybir.dt.int16

    NC1 = TILES * (CA1 + CB1)
    NC2 = TILES * (CA2 + CB2)
    W1 = NC1 * 8  # idx cols (TILE/16 per chunk)
    W2 = NC2 * 8
    CMAX = max(CA1, CB1, CA2, CB2)

    nc = bacc.Bacc("TRN2", target_bir_lowering=False, debug=False,
                   num_devices=N_CORES, num_swdge_queues=4)

    # ---- I/O tensors ----
    xg = nc.dram_tensor("xg", [n_lo + n_hi, din], bf16, kind="ExternalInput")
    xT_d = nc.dram_tensor("xT", [din, SHARD], bf16, kind="ExternalInput")
    idx1_d = nc.dram_tensor("idx1", [128, W1], i16, kind="ExternalInput")
    idx2_d = nc.dram_tensor("idx2", [128, W2], i16, kind="ExternalInput")
    ds1_d = nc.dram_tensor("ds1", [128, NC1], bf16, kind="ExternalInput")
    ds2_d = nc.dram_tensor("ds2", [128, NC2], bf16, kind="ExternalInput")
    rb_d = nc.dram_tensor("rb", [128, SHARD], bf16, kind="ExternalInput")
    rbT_d = nc.dram_tensor("rbT", [128, TILES], f32, kind="ExternalInput")
    w1lT_d = nc.dram_tensor("w1lT", [din, dh], bf16, kind="ExternalInput")
    w1rT_d = nc.dram_tensor("w1rT", [din, dh], bf16, kind="ExternalInput")
    w2lT_d = nc.dram_tensor("w2lT", [128, dh // 128, dout], bf16, kind="ExternalInput")
    w2rT_d = nc.dram_tensor("w2rT", [128, dh // 128, dout], bf16, kind="ExternalInput")
    b1_d = nc.dram_tensor("b1", [128, dh // 128], f32, kind="ExternalInput")
    b2r_d = nc.dram_tensor("b2r", [1, dout], bf16, kind="ExternalInput")
    ones_d = nc.dram_tensor("ones1", [1, 128], bf16, kind="ExternalInput")
    iota_d = nc.dram_tensor("iota", [128, CMAX, 128], bf16, kind="ExternalInput")
    outN_d = nc.dram_tensor("outN", [SHARD, dout], f32, kind="ExternalOutput")

    # internal DRAM
    gl_lo = nc.dram_tensor("gl_lo", [LO_ROWS, dout], bf16)
    gl_hi = nc.dram_tensor("gl_hi", [HI_ROWS, dout], bf16)
    _aspace = "Shared" if shared_g else None
    gf_lo = nc.dram_tensor("gf_lo", [N_CORES * LO_ROWS, dout], bf16,
                           addr_space=_aspace)
    gf_hi = nc.dram_tensor("gf_hi", [N_CORES * HI_ROWS, dout], bf16,
                           addr_space=_aspace)

    NH = dh // 128  # h halves (2)

    with tile.TileContext(nc) as tc:
        with (
            tc.tile_pool(name="per", bufs=1) as per,       # persistent SBUF
            tc.tile_pool(name="gath", bufs=3) as gpool,    # gather buffers
            tc.tile_pool(name="rt", bufs=3) as rpool,      # one-hot R tiles
            tc.tile_pool(name="mt", bufs=2) as mpool,      # meanT / evict tiles
            tc.tile_pool(name="stg", bufs=3) as spool,     # staging for DRAM writes
            tc.tile_pool(name="ps_seg", bufs=2, space="PSUM") as ps_seg,
            tc.tile_pool(name="ps_h", bufs=2, space="PSUM") as ps_h,
            tc.tile_pool(name="ps_g", bufs=2, space="PSUM") as ps_g,
        ):
            # ---- persistent loads ----
            xT = per.tile([din, SHARD], bf16)
            idx = per.tile([128, max(W1, W2)], i16)  # idx1, then idx2
            ds1 = per.tile([128, NC1], bf16)
            ds2 = per.tile([128, NC2], bf16)
            rb = per.tile([128, SHARD], bf16)
            rbT = per.tile([128, TILES], f32)
            w1lT = per.tile([din, dh], bf16)
            w1rT = per.tile([din, dh], bf16)
            w2lT = per.tile([128, NH, dout], bf16)
            w2rT = per.tile([128, NH, dout], bf16)
            b1 = per.tile([128, NH], f32)
            b2r = per.tile([1, dout], bf16)
            ones1 = per.tile([1, 128], bf16)
            iota = per.tile([128, CMAX, 128], bf16)
            HT = per.tile([128, NH, SHARD], bf16)

            # idx1 first: the opening gathers wait on it, and the sync FIFO
            # loads in order.  Then the R-build/H inputs, then the rest.
            nc.sync.dma_start(idx[:, 0:W1], idx1_d[:])
            for t_sb, t_dr in [(ds1, ds1_d), (iota, iota_d), (rb, rb_d),
                               (w1lT, w1lT_d), (w1rT, w1rT_d), (b1, b1_d),
                               (xT, xT_d), (w2lT, w2lT_d), (w2rT, w2rT_d),
                               (ds2, ds2_d), (rbT, rbT_d),
                               (b2r, b2r_d), (ones1, ones_d)]:
                nc.sync.dma_start(t_sb[:], t_dr[:])

            nc.gpsimd.load_library(mlp)

            xg_lo = xg[0:n_lo, :]
            xg_hi = xg[n_lo:n_lo + n_hi, :]

            # ================= Stage A: layer 1 + H + g =================
            a_bufs = {}

            def _issue_a(S):
                mA = gpool.tile([128, SUPER * CA1, din], bf16, tag="mA")
                mB = gpool.tile([128, SUPER * CB1, din], bf16, tag="mB")
                a_bufs[S] = (mA, mB)
                ca_cols = SUPER * CA1 * 8
                cb_cols = SUPER * CB1 * 8
                col0 = S * (ca_cols + cb_cols)
                for buf, nch, src_ap, c0 in [(mA, SUPER * CA1, xg_lo, col0),
                                             (mB, SUPER * CB1, xg_hi, col0 + ca_cols)]:
                    for q0 in range(0, nch, 8):
                        n = min(8, nch - q0)
                        nc.gpsimd.dma_gather(
                            buf[:, q0:q0 + n, :], src_ap,
                            idx[:, c0 + q0 * 8:c0 + (q0 + n) * 8],
                            n * TILE, n * TILE, din)

            rq1 = {}

            def _build_r1(t):
                S, t0 = divmod(t, SUPER)
                gc0 = S * SUPER * (CA1 + CB1)
                rs = []
                for g, (CC, base) in enumerate(
                        [(CA1, gc0), (CB1, gc0 + SUPER * CA1)]):
                    gcs = base + t0 * CC
                    R = rpool.tile([128, CC, 128], bf16,
                                   tag=("RA" if g == 0 else "RB"))
                    nc.vector.tensor_tensor(
                        R[:], iota[:, 0:CC, :],
                        ds1[:, gcs:gcs + CC].broadcast_to([128, CC, 128]),
                        mybir.AluOpType.is_equal)
                    rs.append(R)
                rq1[t] = rs

            import concourse.mybir as _mb

            def _issue_c(S, which, bufs_by_S):
                ca_cols = SUPER * CA2 * 8
                cb_cols = SUPER * CB2 * 8
                col0 = S * (ca_cols + cb_cols)
                if which == "A":
                    mA2 = gpool.tile([128, SUPER * CA2, dout], bf16, tag="mA")
                    bufs_by_S.setdefault(S, {})["A"] = mA2
                    nch, src_ap, c0, buf = SUPER * CA2, gf_lo[:], col0, mA2
                else:
                    mB2 = gpool.tile([128, SUPER * CB2, dout], bf16, tag="mB")
                    bufs_by_S.setdefault(S, {})["B"] = mB2
                    nch, src_ap, c0, buf = (SUPER * CB2, gf_hi[:],
                                            col0 + ca_cols, mB2)
                for q0 in range(0, nch, 8):
                    n = min(8, nch - q0)
                    nc.gpsimd.dma_gather(
                        buf[:, q0:q0 + n, :], src_ap,
                        idx[:, c0 + q0 * 8:c0 + (q0 + n) * 8],
                        n * TILE, n * TILE, dout)

            _c_bufs = {}
            _issue_a(0)
            _issue_a(1)
            _build_r1(0)
            _build_r1(1)
            for t in range(TILES):
                S, t0 = divmod(t, SUPER)
                if t0 == 0 and S + 2 < N_SUPER:
                    _issue_a(S + 2)
                if t0 == 0 and S == N_SUPER - 3 and do_cc:
                    # every stage-A gather is already issued (depth-2
                    # prefetch), so AG-lo here blocks nothing and its
                    # transfer hides behind the last three supertiles.
                    nc.gpsimd.collective_compute(
                        "AllGather", _mb.AluOpType.bypass,
                        replica_groups=[list(range(N_CORES))],
                        ins=[gl_lo.ap().opt()], outs=[gf_lo.ap().opt()])
                if t + 2 < TILES:
                    _build_r1(t + 2)
                mA, mB = a_bufs[S]
                # segment-sum split over two PSUM banks so consecutive
                # accumulating matmuls overlap (same-bank chains serialize).
                psS0 = ps_seg.tile([128, 128], f32, tag="psS0")
                psS1 = ps_seg.tile([128, 128], f32, tag="psS1")
                RA, RB = rq1.pop(t)
                mms = ([(mA, t0 * CA1 + k, RA, k) for k in range(CA1)]
                       + [(mB, t0 * CB1 + k, RB, k) for k in range(CB1)])
                banks = [psS0, psS1]
                nb = [sum(1 for i in range(len(mms)) if i % 2 == b)
                      for b in range(2)]
                cnt = [0, 0]
                for i, (buf, c, R, k) in enumerate(mms):
                    b = i % 2
                    nc.tensor.matmul(banks[b][:], lhsT=buf[:, c, :],
                                     rhs=R[:, k, :], start=(cnt[b] == 0),
                                     stop=(cnt[b] == nb[b] - 1))
                    cnt[b] += 1
                # meanT = (psS0 + psS1) * rdeg  (ACT evicts bank1, DVE fuses)
                s1 = mpool.tile([128, 128], f32, tag="s1")
                nc.scalar.activation(s1[:], psS1[:],
                                     mybir.ActivationFunctionType.Copy)
                ssum = mpool.tile([128, 128], f32, tag="ssum")
                nc.vector.tensor_tensor(ssum[:], psS0[:], s1[:],
                                        mybir.AluOpType.add)
                meanT = mpool.tile([128, 128], bf16, tag="meanT")
                nc.vector.tensor_tensor(
                    meanT[:], ssum[:], rb[:, t * TILE:(t + 1) * TILE],
                    mybir.AluOpType.mult)
                # H^T halves
                for j in range(NH):
                    psH = ps_h.tile([128, 128], f32, tag="psH")
                    nc.tensor.matmul(psH[:], lhsT=w1lT[:, j * 128:(j + 1) * 128],
                                     rhs=meanT[:], start=True, stop=False)
                    nc.tensor.matmul(psH[:], lhsT=w1rT[:, j * 128:(j + 1) * 128],
                                     rhs=xT[:, t * TILE:(t + 1) * TILE],
                                     start=False, stop=True)
                    nc.scalar.activation(HT[:, j, t * TILE:(t + 1) * TILE], psH[:],
                                         mybir.ActivationFunctionType.Relu,
                                         bias=b1[:, j:j + 1])
                # g tile (node-major)
                psG = ps_g.tile([128, 128], f32, tag="psG")
                for j in range(NH):
                    nc.tensor.matmul(psG[:], lhsT=HT[:, j, t * TILE:(t + 1) * TILE],
                                     rhs=w2lT[:, j, :], start=(j == 0),
                                     stop=(j == NH - 1))
                gT = spool.tile([128, dout], bf16, tag="gT")
                nc.scalar.activation(gT[:], psG[:],
                                     mybir.ActivationFunctionType.Copy)
                row = t * TILE
                if row < LO_ROWS:
                    dst = gl_lo[row:row + TILE, :]
                else:
                    dst = gl_hi[row - LO_ROWS:row - LO_ROWS + TILE, :]
                nc.sync.dma_start(dst, gT[:])

            # idx buffer is free of layer-1 readers once stage-A gathers are
            # issued; load the layer-2 index stream (overlaps the AllGather).
            nc.sync.dma_start(idx[:, 0:W2], idx2_d[:])
            if do_c:
                _issue_c(0, "A", _c_bufs)
                _issue_c(1, "A", _c_bufs)

            # ================= Stage C: layer 2 (node-major) =================
            rq2 = {}

            def _build_r2(t):
                S, t0 = divmod(t, SUPER)
                gc0 = S * SUPER * (CA2 + CB2)
                rs = []
                for g, (CC, base) in enumerate(
                        [(CA2, gc0), (CB2, gc0 + SUPER * CA2)]):
                    gcs = base + t0 * CC
                    R = rpool.tile([128, CC, 128], bf16,
                                   tag=("RA2" if g == 0 else "RB2"))
                    nc.vector.tensor_tensor(
                        R[:], iota[:, 0:CC, :],
                        ds2[:, gcs:gcs + CC].broadcast_to([128, CC, 128]),
                        mybir.AluOpType.is_equal)
                    rs.append(R)
                rq2[t] = rs

            if do_cc:
                nc.gpsimd.collective_compute(
                    "AllGather", _mb.AluOpType.bypass,
                    replica_groups=[list(range(N_CORES))],
                    ins=[gl_hi.ap().opt()], outs=[gf_hi.ap().opt()])
            if do_c:
                _issue_c(2, "A", _c_bufs)
                _issue_c(0, "B", _c_bufs)
                _build_r2(0)
                _build_r2(1)
            for t in (range(TILES) if do_c else []):
                S, t0 = divmod(t, SUPER)
                if t0 == 0:
                    if S + 3 < N_SUPER:
                        _issue_c(S + 3, "A", _c_bufs)
                    if S + 1 < N_SUPER:
                        _issue_c(S + 1, "B", _c_bufs)
                if t + 2 < TILES:
                    _build_r2(t + 2)
                mA = _c_bufs[S]["A"]
                mB = _c_bufs[S]["B"]
                # psR[d, f] = H_d @ W2_r^T + b2 issued FIRST: its inputs are
                # always ready, so TensorE fills gather-data waits with it.
                psR = ps_seg.tile([128, 128], f32, tag="psS0")
                for j in range(NH):
                    nc.tensor.matmul(psR[:], lhsT=HT[:, j, t * TILE:(t + 1) * TILE],
                                     rhs=w2rT[:, j, :],
                                     start=(j == 0), stop=False)
                nc.tensor.matmul(psR[:], lhsT=ones1[:], rhs=b2r[:],
                                 start=False, stop=True)
                # psO[d, f] = segment-sum of g[src] (node-major via lhsT=R),
                # split across two PSUM banks (tags reused from stage A).
                psO0 = ps_h.tile([128, 128], f32, tag="psH")
                psO1 = ps_g.tile([128, 128], f32, tag="psG")
                RA, RB = rq2.pop(t)
                mms = ([(mA, t0 * CA2 + k, RA, k) for k in range(CA2)]
                       + [(mB, t0 * CB2 + k, RB, k) for k in range(CB2)])
                banks = [psO0, psO1]
                nb = [sum(1 for i in range(len(mms)) if i % 2 == b)
                      for b in range(2)]
                cnt = [0, 0]
                for i, (buf, c, R, k) in enumerate(mms):
                    b = i % 2
                    nc.tensor.matmul(banks[b][:], lhsT=R[:, k, :],
                                     rhs=buf[:, c, :], start=(cnt[b] == 0),
                                     stop=(cnt[b] == nb[b] - 1))
                    cnt[b] += 1
                # out = (psO0 + psO1) * rdeg(d) + psR (scales fused in ScalarE)
                sc0 = mpool.tile([128, 128], f32, tag="sc")
                nc.scalar.activation(sc0[:], psO0[:],
                                     mybir.ActivationFunctionType.Copy,
                                     scale=rbT[:, t:t + 1])
                sc1 = mpool.tile([128, 128], f32, tag="sc1")
                nc.scalar.activation(sc1[:], psO1[:],
                                     mybir.ActivationFunctionType.Copy,
                                     scale=rbT[:, t:t + 1])
                u = mpool.tile([128, 128], f32, tag="u")
                nc.vector.tensor_tensor(u[:], sc0[:], sc1[:],
                                        mybir.AluOpType.add)
                oN = spool.tile([128, 128], f32, tag="oN")
                nc.vector.tensor_tensor(oN[:], u[:], psR[:],
                                        mybir.AluOpType.add)
                nc.sync.dma_start(
                    outN_d[t * TILE:(t + 1) * TILE, :], oN[:])

    # Align each gather's SWDGE queue with the DMASW sem lane Tile assigned
    # (sem lane L is locked to one queue; use queue = L % num_queues).
    import re as _re
    for bb in nc.main_func.blocks:
        for ins in bb.instructions:
            if isinstance(ins, mybir.InstDMAGatherAnt):
                lane = None
                si = ins.sync_info
                if si is not None:
                    for upd in list(si.on_update):
                        m = _re.match(r"DMASW(\d+)", getattr(upd, "ant_name", None) or "")
                        if m:
                            lane = int(m.group(1))
                if lane is not None:
                    ins.queue_num = lane % 4
    nc.compile()
    return nc


def kernel(x, edge_index, W1_l, b1_l, W1_r, W2_l, b2_l, W2_r):
    import ml_dtypes
    from concourse.bass_utils import run_bass_kernel_spmd

    x = np.asarray(x, np.float32)
    n_nodes, din = x.shape
    dh = W1_l.shape[0]
    dout = W2_l.shape[0]

    meta = _preprocess(x, edge_index, n_nodes)
    l1, l2 = meta["l1"], meta["l2"]

    n_lo = SPLIT16
    n_hi = n_nodes - SPLIT16
    nc = _build_program(din, dh, dout, l1["CA"], l1["CB"], l2["CA"], l2["CB"],
                        n_lo, n_hi)

    bf = ml_dtypes.bfloat16
    xg = x.astype(bf)
    w1lT = np.ascontiguousarray(np.asarray(W1_l, np.float32).T).astype(bf)  # [din, dh]
    w1rT = np.ascontiguousarray(np.asarray(W1_r, np.float32).T).astype(bf)
    # [dh, dout] -> [128, dh//128, dout]
    w2lT = np.ascontiguousarray(np.asarray(W2_l, np.float32).T).reshape(
        dh // 128, 128, dout).transpose(1, 0, 2).astype(bf)
    w2rT = np.ascontiguousarray(np.asarray(W2_r, np.float32).T).reshape(
        dh // 128, 128, dout).transpose(1, 0, 2).astype(bf)
    b1 = np.ascontiguousarray(
        np.asarray(b1_l, np.float32).reshape(dh // 128, 128).T)  # [128, nh]
    b2r = np.asarray(b2_l, np.float32).reshape(1, dout).astype(bf)
    ones1 = np.ones((1, 128), np.float32).astype(bf)
    CMAX = max(l1["CA"], l1["CB"], l2["CA"], l2["CB"])
    iota = np.ascontiguousarray(np.broadcast_to(
        np.arange(128, dtype=np.float32), (128, CMAX, 128))).astype(bf)

    in_maps = []
    for c in range(N_CORES):
        in_maps.append({
            "xg": xg, "xT": meta["xT"][c],
            "idx1": l1["idx"][c], "idx2": l2["idx"][c],
            "ds1": l1["ds"][c].astype(bf), "ds2": l2["ds"][c].astype(bf),
            "rb": meta["rb"][c], "rbT": meta["rbT"][c],
            "w1lT": w1lT, "w1rT": w1rT, "w2lT": w2lT, "w2rT": w2rT,
            "b1": b1, "b2r": b2r, "ones1": ones1, "iota": iota,
        })

    res = run_bass_kernel_spmd(nc, in_maps, list(range(N_CORES)))

    out = np.empty((n_nodes, dout), np.float32)
    core_of, row_of = meta["core_of"], meta["row_of"]
    outNs = np.stack([np.asarray(res.results[c]["outN"], np.float32)
                      for c in range(N_CORES)])  # [8, SHARD, dout]
    out[:, :] = outNs[core_of, row_of, :]
    return out
